# revision 6
# baseline (speedup 1.0000x reference)
"""Multi-headed self-attention on 8 Trainium2 NeuronCores (Bass/Tile).

Problem: B=8, S=1024, D=1024, H=16 heads (DH=64), fp32.
    qp = q @ Wq.T + bq ; kp = k @ Wk.T + bk ; vp = v @ Wv.T + bv
    out = softmax(Qh Kh^T / sqrt(DH) + maskbias) Vh   (per head, merged)

Sharding: data-parallel over batch — one batch element per core.

Per-core device algorithm (all matmuls in fp32r = tf32-like, 4x faster
than fp32 on the PE at equal storage):
  1. Projections with the contraction dim on partitions. Host pre-
     transposes inputs and weights, so q^T/k^T/v^T [D,S] and W^T [D,D]
     load as plain contiguous DMAs (cast to fp32r in-flight by SWDGE).
       qpT [D,S] = (Wq^T)^T.T @ q^T   (transposed output layout)
       kpT [D,S] likewise
       vp  [S,D] = (v^T).T @ Wv^T     (natural layout), scattered into
         v_aug [S, H*(DH+1)]: per head 64 V columns + one ones column.
  2. Attention per (head, q-chunk): scoresT [k,q] tiles = Kh^T.T @ Qh^T
     with k on partitions. Softmax over k needs no max subtraction
     (logits are O(+-8), fp32 exp is safe): exp via ACT with the mask
     bias as a per-partition bias and scale=1/sqrt(DH).
  3. AV: outT [DH+1, q] = [Vh | 1].T @ expT accumulated over k tiles;
     row DH is the softmax denominator (free via the ones column).
  4. PE-transpose outT 128-col blocks -> [q, DH+1]; per-partition
     reciprocal of col DH; tensor_scalar multiply -> normalized head
     output written straight into the assembled [128, D] output tile.
Heads are processed in pairs: DH=64 so a head pair shares one 128-
partition tile of qpT/kpT; scores matmuls for the pair go to PE row
groups (0,0)/(64,0) and overlap in the array.
"""

import os
import sys

for _p in (
    "/root/.axon_site",
    "/root/.axon_site/_ro/trn_rl_repo",
    "/root/.axon_site/_ro/pypackages",
    "/opt/trn_rl_repo",
):
    if os.path.isdir(_p) and _p not in sys.path:
        sys.path.append(_p)

import numpy as np

import concourse.bass as bass
import concourse.tile as tile
import concourse.mybir as mybir
from concourse import bacc
from concourse.bass_utils import run_bass_kernel_spmd
from concourse.masks import make_identity

B, S, D, H = 8, 1024, 1024, 16
DH = D // H  # 64
N_CORES = 8
P = 128  # partitions

F32 = mybir.dt.float32
F32R = mybir.dt.float32r


def build_bass(s=S, d=D, h=H, debug=False):
    """Build the per-core Bass program. Same program on all 8 cores."""
    dh = d // h
    kt_n = d // P          # contraction tiles (projections)
    ot_n = d // P          # output-feature tiles
    st_n = s // P          # sequence tiles of 128
    ch = 512 if s % 512 == 0 else s   # moving-dim chunk (<= 512, fp32 PSUM bank)
    ch_n = s // ch         # chunks per sequence
    qb_n = ch // P         # 128-q blocks per chunk
    hp_n = P // dh         # heads per 128-partition tile (2)
    vaug_w = h * (dh + 1)  # v_aug width

    nc = bacc.Bacc(
        "TRN2", target_bir_lowering=False, debug=debug, num_devices=N_CORES
    )

    qT = nc.dram_tensor("qT", (d, s), F32, kind="ExternalInput").ap()
    kT = nc.dram_tensor("kT", (d, s), F32, kind="ExternalInput").ap()
    vT = nc.dram_tensor("vT", (d, s), F32, kind="ExternalInput").ap()
    wqT = nc.dram_tensor("wqT", (d, d), F32, kind="ExternalInput").ap()
    wkT = nc.dram_tensor("wkT", (d, d), F32, kind="ExternalInput").ap()
    wvT = nc.dram_tensor("wvT", (d, d), F32, kind="ExternalInput").ap()
    bqT = nc.dram_tensor("bqT", (P, ot_n), F32, kind="ExternalInput").ap()
    bkT = nc.dram_tensor("bkT", (P, ot_n), F32, kind="ExternalInput").ap()
    # per head: [bv head-slice (dh) | 1.0] — the trailing 1.0 seeds the
    # ones column of v_aug (softmax denominator trick)
    bvB = nc.dram_tensor("bvB", (P, h * (d // h + 1)), F32, kind="ExternalInput").ap()
    mb = nc.dram_tensor("mb", (P, st_n), F32, kind="ExternalInput").ap()
    outd = nc.dram_tensor("out", (s, d), F32, kind="ExternalOutput").ap()

    with tile.TileContext(nc) as tc:
        with tc.tile_pool(name="singles", bufs=1) as singles:
            ident = singles.tile([P, P], F32)
            make_identity(nc, ident)
            mb_t = singles.tile([P, st_n], F32)
            nc.sync.dma_start(out=mb_t, in_=mb)
            bq_t = singles.tile([P, ot_n], F32)
            nc.sync.dma_start(out=bq_t, in_=bqT)
            bk_t = singles.tile([P, ot_n], F32)
            nc.sync.dma_start(out=bk_t, in_=bkT)
            bv_t = singles.tile([P, vaug_w], F32)
            nc.sync.dma_start(out=bv_t, in_=bvB)

            # ---- persistent phase-A outputs ----
            with tc.tile_pool(name="projout", bufs=2 * ot_n) as projout, \
                 tc.tile_pool(name="vaug", bufs=st_n) as vaugp:
                qp_tiles = []
                kp_tiles = []
                vaug_tiles = []

                # ================= Phase A: projections =================
                with tc.tile_pool(name="wpool", bufs=kt_n + 1) as wpool, \
                     tc.tile_pool(name="inpool", bufs=kt_n + 1) as inpool, \
                     tc.tile_pool(name="ppsum", bufs=4, space="PSUM") as ppsum:

                    def load_tiles(pool, dram, tag):
                        tiles = []
                        for kt in range(kt_n):
                            t = pool.tile([P, dram.shape[1]], F32R, tag=tag)
                            nc.gpsimd.dma_start(
                                out=t, in_=dram[kt * P:(kt + 1) * P, :]
                            )
                            tiles.append(t)
                        return tiles

                    # --- qpT / kpT (transposed-output projections) ---
                    for name, ind, wd, bias_t, out_list in (
                        ("qp", qT, wqT, bq_t, qp_tiles),
                        ("kp", kT, wkT, bk_t, kp_tiles),
                    ):
                        w_tiles = load_tiles(wpool, wd, "w")
                        x_tiles = load_tiles(inpool, ind, "in")
                        for ot in range(ot_n):
                            po = projout.tile([P, s], F32R, tag="projout")
                            out_list.append(po)
                            for sc in range(ch_n):
                                ps = ppsum.tile([P, ch], F32, tag="ppsum")
                                for kt in range(kt_n):
                                    nc.tensor.matmul(
                                        ps,
                                        w_tiles[kt][:, ot * P:(ot + 1) * P],
                                        x_tiles[kt][:, sc * ch:(sc + 1) * ch],
                                        start=(kt == 0),
                                        stop=(kt == kt_n - 1),
                                    )
                                nc.vector.tensor_scalar_add(
                                    po[:, sc * ch:(sc + 1) * ch],
                                    ps,
                                    bias_t[:, ot:ot + 1],
                                )

                    # --- vp -> v_aug (natural layout + ones columns) ---
                    w_tiles = load_tiles(wpool, wvT, "w")
                    x_tiles = load_tiles(inpool, vT, "in")
                    oc_n = d // ch
                    hpc = ch // (dh + 1) if False else None  # unused
                    bv_g = bv_t.rearrange("p (g c) -> p g c", c=dh + 1)
                    for st in range(st_n):
                        va = vaugp.tile([P, vaug_w], F32R, tag="vaug")
                        vaug_tiles.append(va)
                        va_g = va.rearrange("p (g c) -> p g c", c=dh + 1)
                        for oc in range(oc_n):
                            ps = ppsum.tile([P, ch], F32, tag="ppsum")
                            for kt in range(kt_n):
                                nc.tensor.matmul(
                                    ps,
                                    x_tiles[kt][:, st * P:(st + 1) * P],
                                    w_tiles[kt][:, oc * ch:(oc + 1) * ch],
                                    start=(kt == 0),
                                    stop=(kt == kt_n - 1),
                                )
                            g0 = oc * (ch // dh)  # first head group in chunk
                            gn = ch // dh
                            nc.vector.tensor_tensor(
                                out=va_g[:, g0:g0 + gn, 0:dh],
                                in0=ps.rearrange("p (g c) -> p g c", c=dh),
                                in1=bv_g[:, g0:g0 + gn, 0:dh],
                                op=mybir.AluOpType.add,
                            )
                        nc.vector.tensor_copy(
                            va_g[:, :, dh:dh + 1], bv_g[:, :, dh:dh + 1]
                        )

                # ================= Phase B: attention =================
                with tc.tile_pool(name="expp", bufs=2 * st_n + 2) as expp, \
                     tc.tile_pool(name="otsp", bufs=4) as otsp, \
                     tc.tile_pool(name="finalp", bufs=2 * qb_n) as finalp, \
                     tc.tile_pool(name="rcpp", bufs=8) as rcpp, \
                     tc.tile_pool(name="spsum", bufs=4, space="PSUM") as spsum, \
                     tc.tile_pool(name="opsum", bufs=2, space="PSUM") as opsum, \
                     tc.tile_pool(name="tpsum", bufs=2, space="PSUM") as tpsum:

                    for qc in range(ch_n):  # q chunk of `ch` columns
                        finals = []
                        for qb in range(qb_n):
                            fin = finalp.tile([P, d], F32, tag="final", name=f"fin_{qc}_{qb}")
                            finals.append(fin)
                        for h2 in range(h // hp_n):  # head pairs
                            # scores + exp for the pair, row-group packed
                            exp_tiles = {}
                            for kt in range(st_n):
                                for hp in range(hp_n):
                                    hh = h2 * hp_n + hp
                                    sc_ps = spsum.tile([P, ch], F32, tag="spsum")
                                    nc.tensor.matmul(
                                        sc_ps,
                                        kp_tiles[h2][
                                            hp * dh:(hp + 1) * dh,
                                            kt * P:(kt + 1) * P,
                                        ],
                                        qp_tiles[h2][
                                            hp * dh:(hp + 1) * dh,
                                            qc * ch:(qc + 1) * ch,
                                        ],
                                        start=True,
                                        stop=True,
                                        tile_position=(hp * dh, 0),
                                    )
                                    et = expp.tile([P, ch], F32R, tag="exp")
                                    nc.scalar.activation(
                                        et,
                                        sc_ps,
                                        mybir.ActivationFunctionType.Exp,
                                        bias=mb_t[:, kt:kt + 1],
                                        scale=1.0 / float(np.sqrt(dh)),
                                    )
                                    exp_tiles[(hh, kt)] = et
                            # AV + normalize per head of the pair
                            for hp in range(hp_n):
                                hh = h2 * hp_n + hp
                                ot_ps = opsum.tile([dh + 1, ch], F32, tag="opsum")
                                for kt in range(st_n):
                                    nc.tensor.matmul(
                                        ot_ps,
                                        vaug_tiles[kt][
                                            :, hh * (dh + 1):(hh + 1) * (dh + 1)
                                        ],
                                        exp_tiles[(hh, kt)],
                                        start=(kt == 0),
                                        stop=(kt == st_n - 1),
                                    )
                                ots = otsp.tile([dh + 1, ch], F32, tag="ots")
                                nc.vector.tensor_copy(ots, ot_ps)
                                for qb in range(qb_n):
                                    tr = tpsum.tile([P, dh + 1], F32, tag="tpsum")
                                    nc.tensor.transpose(
                                        tr,
                                        ots[:, qb * P:(qb + 1) * P],
                                        ident[0:dh + 1, 0:dh + 1],
                                    )
                                    rcp = rcpp.tile([P, 1], F32, tag="rcp")
                                    nc.vector.reciprocal(rcp, tr[:, dh:dh + 1])
                                    nc.vector.tensor_scalar_mul(
                                        finals[qb][:, hh * dh:(hh + 1) * dh],
                                        tr[:, 0:dh],
                                        rcp,
                                    )
                        for qb in range(qb_n):
                            row0 = qc * ch + qb * P
                            nc.sync.dma_start(
                                out=outd[row0:row0 + P, :], in_=finals[qb]
                            )

    return nc


_CACHE = {}


def _get_compiled():
    if "nc" not in _CACHE:
        nc = build_bass()
        nc.compile()
        _CACHE["nc"] = nc
    return _CACHE["nc"]


def kernel(q, k, v, mask, Wq, bq, Wk, bk, Wv, bv):
    q = np.asarray(q, dtype=np.float32)
    k = np.asarray(k, dtype=np.float32)
    v = np.asarray(v, dtype=np.float32)
    mask = np.asarray(mask, dtype=np.float32)
    Wq = np.asarray(Wq, dtype=np.float32)
    Wk = np.asarray(Wk, dtype=np.float32)
    Wv = np.asarray(Wv, dtype=np.float32)
    bq = np.asarray(bq, dtype=np.float32)
    bk = np.asarray(bk, dtype=np.float32)
    bv = np.asarray(bv, dtype=np.float32)

    nc = _get_compiled()

    ot_n = D // P
    st_n = S // P
    # shared (per-core identical) host-side layout prep
    wqT = np.ascontiguousarray(Wq.T)
    wkT = np.ascontiguousarray(Wk.T)
    wvT = np.ascontiguousarray(Wv.T)
    bqT = np.ascontiguousarray(bq.reshape(ot_n, P).T)
    bkT = np.ascontiguousarray(bk.reshape(ot_n, P).T)
    # [bv head-slice | 1.0] per head, broadcast across partitions
    bv_aug = np.concatenate(
        [np.concatenate([bv.reshape(H, DH), np.ones((H, 1), np.float32)], axis=1).reshape(-1)]
    ).astype(np.float32)
    bvB = np.ascontiguousarray(np.broadcast_to(bv_aug, (P, H * (DH + 1))))

    in_maps = []
    for b in range(B):
        mbias = (-10000.0 * (1.0 - mask[b])).astype(np.float32)
        in_maps.append({
            "qT": np.ascontiguousarray(q[b].T),
            "kT": np.ascontiguousarray(k[b].T),
            "vT": np.ascontiguousarray(v[b].T),
            "wqT": wqT,
            "wkT": wkT,
            "wvT": wvT,
            "bqT": bqT,
            "bkT": bkT,
            "bvB": bvB,
            "mb": np.ascontiguousarray(mbias.reshape(st_n, P).T),
        })

    res = run_bass_kernel_spmd(nc, in_maps, core_ids=list(range(N_CORES)))
    out = np.stack([res.results[b]["out"] for b in range(B)], axis=0)
    return out.astype(np.float32)


# revision 16
# speedup vs baseline: 1.6143x; 1.6143x over previous
"""Multi-headed self-attention on 8 Trainium2 NeuronCores (Bass/Tile).

Problem: B=8, S=1024, D=1024, H=16 heads (DH=64), fp32.
    qp = q @ Wq.T + bq ; kp = k @ Wk.T + bk ; vp = v @ Wv.T + bv
    out = softmax(Qh Kh^T / sqrt(DH) + maskbias) Vh   (per head, merged)

Sharding: data-parallel over batch — one batch element per core.

Per-core device algorithm (all matmuls in fp32r = tf32-like, 4x faster
than fp32 on the PE at equal storage):
  1. Projections with the contraction dim on partitions. Host pre-
     transposes inputs and weights, so q^T/k^T/v^T [D,S] and W^T [D,D]
     load as plain contiguous DMAs (cast to fp32r in-flight by SWDGE).
       qpT [D,S] = (Wq^T)^T.T @ q^T   (transposed output layout)
       kpT [D,S] likewise
       vp  [S,D] = (v^T).T @ Wv^T     (natural layout), scattered into
         v_aug [S, H*(DH+1)]: per head 64 V columns + one ones column.
  2. Attention per (head, q-chunk): scoresT [k,q] tiles = Kh^T.T @ Qh^T
     with k on partitions. Softmax over k needs no max subtraction
     (logits are O(+-8), fp32 exp is safe): exp via ACT with the mask
     bias as a per-partition bias and scale=1/sqrt(DH).
  3. AV: outT [DH+1, q] = [Vh | 1].T @ expT accumulated over k tiles;
     row DH is the softmax denominator (free via the ones column).
  4. Transpose outT 128-col blocks -> [q, DH+1] via a REGULAR fp32r
     matmul against an identity (transpose-mode interleave stalls fp32r
     streams ~1.3us/mm); per-partition reciprocal of col DH;
     tensor_scalar multiply -> normalized head output written straight
     into the assembled [128, D] output tile.

fp32r HW quirks found by microbenchmark (honor these):
  - moving dim (output free size) must be EVEN -> identity matmul uses
    N=66, not 65.
  - tile_position / base_partition=64 operands stall ~1.5us per matmul
    (and blocked tile_position streams can hang the device). All score
    matmuls therefore use full K=128 with ZERO-PADDED per-head K tiles:
    head in one 64-row half, zeros in the other; the matching qpT pair
    tile rows are annihilated by the zeros. Uniform base-0 K=128
    streams measure ~232 ns/mm.
"""

import os
import sys

for _p in (
    "/root/.axon_site",
    "/root/.axon_site/_ro/trn_rl_repo",
    "/root/.axon_site/_ro/pypackages",
    "/opt/trn_rl_repo",
):
    if os.path.isdir(_p) and _p not in sys.path:
        sys.path.append(_p)

import numpy as np

import concourse.bass as bass
import concourse.tile as tile
import concourse.mybir as mybir
from concourse import bacc
from concourse.bass_utils import run_bass_kernel_spmd
from concourse.masks import make_identity

B, S, D, H = 8, 1024, 1024, 16
DH = D // H  # 64
N_CORES = 8
P = 128  # partitions

F32 = mybir.dt.float32
F32R = mybir.dt.float32r


def build_bass(s=S, d=D, h=H, debug=False):
    """Build the per-core Bass program. Same program on all 8 cores."""
    dh = d // h
    kt_n = d // P          # contraction tiles (projections)
    ot_n = d // P          # output-feature tiles
    st_n = s // P          # sequence tiles of 128
    ch = 512 if s % 512 == 0 else s   # moving-dim chunk (<= 512, fp32 PSUM bank)
    ch_n = s // ch         # chunks per sequence
    qb_n = ch // P         # 128-q blocks per chunk
    hp_n = P // dh         # heads per 128-partition tile (2)
    vaug_w = h * (dh + 1)  # v_aug width

    nc = bacc.Bacc(
        "TRN2", target_bir_lowering=False, debug=debug, num_devices=N_CORES
    )

    qT = nc.dram_tensor("qT", (d, s), F32, kind="ExternalInput").ap()
    kT = nc.dram_tensor("kT", (d, s), F32, kind="ExternalInput").ap()
    vT = nc.dram_tensor("vT", (d, s), F32, kind="ExternalInput").ap()
    wqT = nc.dram_tensor("wqT", (d, d), F32, kind="ExternalInput").ap()
    wkT = nc.dram_tensor("wkT", (d, d), F32, kind="ExternalInput").ap()
    wvT = nc.dram_tensor("wvT", (d, d), F32, kind="ExternalInput").ap()
    bqT = nc.dram_tensor("bqT", (P, ot_n), F32, kind="ExternalInput").ap()
    bkT = nc.dram_tensor("bkT", (P, ot_n), F32, kind="ExternalInput").ap()
    # per head: [bv head-slice (dh) | 1.0] — the trailing 1.0 seeds the
    # ones column of v_aug (softmax denominator trick)
    bvB = nc.dram_tensor("bvB", (P, h * (d // h + 1)), F32, kind="ExternalInput").ap()
    mb = nc.dram_tensor("mb", (P, st_n), F32, kind="ExternalInput").ap()
    zpad = nc.dram_tensor("zpad", (1, s), F32, kind="ExternalInput").ap()
    outd = nc.dram_tensor("out", (s, d), F32, kind="ExternalOutput").ap()

    def zpad_bcast(parts):
        return bass.AP(tensor=zpad.tensor, offset=0, ap=[[0, parts], [1, s]])

    with tile.TileContext(nc) as tc:
        with tc.tile_pool(name="singles", bufs=1) as singles:
            ident = singles.tile([P, P], F32)
            make_identity(nc, ident)
            # fp32r copy of the identity for the transpose matmuls
            idr = singles.tile([P, P], F32R)
            nc.vector.tensor_copy(idr, ident)
            mb_t = singles.tile([P, st_n], F32)
            nc.sync.dma_start(out=mb_t, in_=mb)
            bq_t = singles.tile([P, ot_n], F32)
            nc.sync.dma_start(out=bq_t, in_=bqT)
            bk_t = singles.tile([P, ot_n], F32)
            nc.sync.dma_start(out=bk_t, in_=bkT)
            bv_t = singles.tile([P, vaug_w], F32)
            nc.sync.dma_start(out=bv_t, in_=bvB)

            # ---- persistent phase-A outputs ----
            # qp: ot_n pair-tiles [128, s]; kp: h zero-padded head tiles
            with tc.tile_pool(name="projout", bufs=ot_n + h) as projout, \
                 tc.tile_pool(name="vaug", bufs=st_n) as vaugp:
                qp_tiles = []
                kp_tiles = []
                vaug_tiles = []

                # ================= Phase A: projections =================
                with tc.tile_pool(name="wpool", bufs=kt_n + 1) as wpool, \
                     tc.tile_pool(name="inpool", bufs=kt_n + 1) as inpool, \
                     tc.tile_pool(name="ppsum", bufs=4, space="PSUM") as ppsum:

                    def load_tiles(pool, dram, tag):
                        tiles = []
                        for kt in range(kt_n):
                            t = pool.tile([P, dram.shape[1]], F32R, tag=tag)
                            nc.gpsimd.dma_start(
                                out=t, in_=dram[kt * P:(kt + 1) * P, :]
                            )
                            tiles.append(t)
                        return tiles

                    # --- qpT (pair tiles, transposed-output projection) ---
                    w_tiles = load_tiles(wpool, wqT, "w")
                    x_tiles = load_tiles(inpool, qT, "in")
                    for ot in range(ot_n):
                        po = projout.tile([P, s], F32R, tag="projout",
                                          name=f"qp_{ot}")
                        qp_tiles.append(po)
                        for sc in range(ch_n):
                            ps = ppsum.tile([P, ch], F32, tag="ppsum")
                            for kt in range(kt_n):
                                nc.tensor.matmul(
                                    ps,
                                    w_tiles[kt][:, ot * P:(ot + 1) * P],
                                    x_tiles[kt][:, sc * ch:(sc + 1) * ch],
                                    start=(kt == 0),
                                    stop=(kt == kt_n - 1),
                                )
                            nc.vector.tensor_scalar_add(
                                po[:, sc * ch:(sc + 1) * ch],
                                ps,
                                bq_t[:, ot:ot + 1],
                            )

                    # --- kpT (per-head zero-padded tiles) ---
                    w_tiles = load_tiles(wpool, wkT, "w")
                    x_tiles = load_tiles(inpool, kT, "in")
                    for ot in range(ot_n):
                        heads = []
                        for hp in range(hp_n):
                            kpo = projout.tile([P, s], F32R, tag="projout",
                                               name=f"kp_{ot}_{hp}")
                            kp_tiles.append(kpo)
                            heads.append(kpo)
                            # zero the unused 64-row half (SWDGE bcast + cast)
                            pad0 = 0 if hp else dh
                            nc.gpsimd.dma_start(
                                out=kpo[pad0:pad0 + (P - dh), :],
                                in_=zpad_bcast(P - dh),
                            )
                        for sc in range(ch_n):
                            ps = ppsum.tile([P, ch], F32, tag="ppsum")
                            for kt in range(kt_n):
                                nc.tensor.matmul(
                                    ps,
                                    w_tiles[kt][:, ot * P:(ot + 1) * P],
                                    x_tiles[kt][:, sc * ch:(sc + 1) * ch],
                                    start=(kt == 0),
                                    stop=(kt == kt_n - 1),
                                )
                            for hp in range(hp_n):
                                rows = slice(hp * dh, (hp + 1) * dh)
                                nc.vector.tensor_scalar_add(
                                    heads[hp][rows, sc * ch:(sc + 1) * ch],
                                    ps[rows, :],
                                    bk_t[rows, ot:ot + 1],
                                )

                    # --- vp -> v_aug (natural layout + ones columns) ---
                    w_tiles = load_tiles(wpool, wvT, "w")
                    x_tiles = load_tiles(inpool, vT, "in")
                    oc_n = d // ch
                    hpc = ch // (dh + 1) if False else None  # unused
                    bv_g = bv_t.rearrange("p (g c) -> p g c", c=dh + 1)
                    for st in range(st_n):
                        va = vaugp.tile([P, vaug_w], F32R, tag="vaug")
                        vaug_tiles.append(va)
                        va_g = va.rearrange("p (g c) -> p g c", c=dh + 1)
                        for oc in range(oc_n):
                            ps = ppsum.tile([P, ch], F32, tag="ppsum")
                            for kt in range(kt_n):
                                nc.tensor.matmul(
                                    ps,
                                    x_tiles[kt][:, st * P:(st + 1) * P],
                                    w_tiles[kt][:, oc * ch:(oc + 1) * ch],
                                    start=(kt == 0),
                                    stop=(kt == kt_n - 1),
                                )
                            g0 = oc * (ch // dh)  # first head group in chunk
                            gn = ch // dh
                            nc.vector.tensor_tensor(
                                out=va_g[:, g0:g0 + gn, 0:dh],
                                in0=ps.rearrange("p (g c) -> p g c", c=dh),
                                in1=bv_g[:, g0:g0 + gn, 0:dh],
                                op=mybir.AluOpType.add,
                            )
                        nc.vector.tensor_copy(
                            va_g[:, :, dh:dh + 1], bv_g[:, :, dh:dh + 1]
                        )

                # ================= Phase B: attention =================
                with tc.tile_pool(name="expp", bufs=2 * st_n) as expp, \
                     tc.tile_pool(name="otsp", bufs=4) as otsp, \
                     tc.tile_pool(name="finalp", bufs=qb_n + 2) as finalp, \
                     tc.tile_pool(name="rcpp", bufs=8) as rcpp, \
                     tc.tile_pool(name="spsum", bufs=4, space="PSUM") as spsum, \
                     tc.tile_pool(name="opsum", bufs=2, space="PSUM") as opsum, \
                     tc.tile_pool(name="tpsum", bufs=2, space="PSUM") as tpsum:

                    for qc in range(ch_n):  # q chunk of `ch` columns
                        finals = []
                        for qb in range(qb_n):
                            fin = finalp.tile([P, d], F32, tag="final", name=f"fin_{qc}_{qb}")
                            finals.append(fin)
                        for h2 in range(h // hp_n):  # head pairs
                            # scores + exp: full-K128 matmuls against the
                            # zero-padded per-head kp tiles (no tile_position)
                            exp_tiles = {}
                            for hp in range(hp_n):
                                hh = h2 * hp_n + hp
                                for kt in range(st_n):
                                    sc_ps = spsum.tile([P, ch], F32, tag="spsum")
                                    nc.tensor.matmul(
                                        sc_ps,
                                        kp_tiles[hh][:, kt * P:(kt + 1) * P],
                                        qp_tiles[h2][:, qc * ch:(qc + 1) * ch],
                                        start=True,
                                        stop=True,
                                    )
                                    et = expp.tile([P, ch], F32R, tag="exp")
                                    nc.scalar.activation(
                                        et,
                                        sc_ps,
                                        mybir.ActivationFunctionType.Exp,
                                        bias=mb_t[:, kt:kt + 1],
                                        scale=1.0 / float(np.sqrt(dh)),
                                    )
                                    exp_tiles[(hh, kt)] = et
                            # AV + normalize per head of the pair
                            for hp in range(hp_n):
                                hh = h2 * hp_n + hp
                                ot_ps = opsum.tile([dh + 1, ch], F32, tag="opsum")
                                for kt in range(st_n):
                                    nc.tensor.matmul(
                                        ot_ps,
                                        vaug_tiles[kt][
                                            :, hh * (dh + 1):(hh + 1) * (dh + 1)
                                        ],
                                        exp_tiles[(hh, kt)],
                                        start=(kt == 0),
                                        stop=(kt == st_n - 1),
                                    )
                                ots = otsp.tile([dh + 1, ch], F32R, tag="ots")
                                nc.vector.tensor_copy(ots, ot_ps)
                                for qb in range(qb_n):
                                    # transpose via REGULAR fp32r matmul with
                                    # identity; fp32r needs even N -> dh+2
                                    tr = tpsum.tile([P, dh + 2], F32, tag="tpsum")
                                    nc.tensor.matmul(
                                        tr,
                                        ots[:, qb * P:(qb + 1) * P],
                                        idr[0:dh + 1, 0:dh + 2],
                                        start=True,
                                        stop=True,
                                    )
                                    rcp = rcpp.tile([P, 1], F32, tag="rcp")
                                    nc.vector.reciprocal(rcp, tr[:, dh:dh + 1])
                                    nc.vector.tensor_scalar_mul(
                                        finals[qb][:, hh * dh:(hh + 1) * dh],
                                        tr[:, 0:dh],
                                        rcp,
                                    )
                        for qb in range(qb_n):
                            row0 = qc * ch + qb * P
                            nc.sync.dma_start(
                                out=outd[row0:row0 + P, :], in_=finals[qb]
                            )

    return nc


_CACHE = {}


def _get_compiled():
    if "nc" not in _CACHE:
        nc = build_bass()
        nc.compile()
        _CACHE["nc"] = nc
    return _CACHE["nc"]


def kernel(q, k, v, mask, Wq, bq, Wk, bk, Wv, bv):
    q = np.asarray(q, dtype=np.float32)
    k = np.asarray(k, dtype=np.float32)
    v = np.asarray(v, dtype=np.float32)
    mask = np.asarray(mask, dtype=np.float32)
    Wq = np.asarray(Wq, dtype=np.float32)
    Wk = np.asarray(Wk, dtype=np.float32)
    Wv = np.asarray(Wv, dtype=np.float32)
    bq = np.asarray(bq, dtype=np.float32)
    bk = np.asarray(bk, dtype=np.float32)
    bv = np.asarray(bv, dtype=np.float32)

    nc = _get_compiled()

    ot_n = D // P
    st_n = S // P
    # shared (per-core identical) host-side layout prep
    wqT = np.ascontiguousarray(Wq.T)
    wkT = np.ascontiguousarray(Wk.T)
    wvT = np.ascontiguousarray(Wv.T)
    bqT = np.ascontiguousarray(bq.reshape(ot_n, P).T)
    bkT = np.ascontiguousarray(bk.reshape(ot_n, P).T)
    # [bv head-slice | 1.0] per head, broadcast across partitions
    bv_aug = np.concatenate(
        [np.concatenate([bv.reshape(H, DH), np.ones((H, 1), np.float32)], axis=1).reshape(-1)]
    ).astype(np.float32)
    bvB = np.ascontiguousarray(np.broadcast_to(bv_aug, (P, H * (DH + 1))))

    in_maps = []
    for b in range(B):
        mbias = (-10000.0 * (1.0 - mask[b])).astype(np.float32)
        in_maps.append({
            "qT": np.ascontiguousarray(q[b].T),
            "kT": np.ascontiguousarray(k[b].T),
            "vT": np.ascontiguousarray(v[b].T),
            "wqT": wqT,
            "wkT": wkT,
            "wvT": wvT,
            "bqT": bqT,
            "bkT": bkT,
            "bvB": bvB,
            "mb": np.ascontiguousarray(mbias.reshape(st_n, P).T),
            "zpad": np.zeros((1, S), np.float32),
        })

    _CACHE["in_maps"] = in_maps
    res = run_bass_kernel_spmd(nc, in_maps, core_ids=list(range(N_CORES)))
    out = np.stack([res.results[b]["out"] for b in range(B)], axis=0)
    return out.astype(np.float32)


# revision 22
# speedup vs baseline: 1.6859x; 1.0443x over previous
"""Multi-headed self-attention on 8 Trainium2 NeuronCores (Bass/Tile).

Problem: B=8, S=1024, D=1024, H=16 heads (DH=64), fp32.
    qp = q @ Wq.T + bq ; kp = k @ Wk.T + bk ; vp = v @ Wv.T + bv
    out = softmax(Qh Kh^T / sqrt(DH) + maskbias) Vh   (per head, merged)

Sharding: data-parallel over batch — one batch element per core.

Per-core device algorithm (all matmuls in fp32r = tf32-like, 4x faster
than fp32 on the PE at equal storage):
  1. Projections with the contraction dim on partitions. Host pre-
     transposes inputs and weights, so q^T/k^T/v^T [D,S] and W^T [D,D]
     load as plain contiguous DMAs (cast to fp32r in-flight by SWDGE).
       qpT [D,S] = (Wq^T)^T.T @ q^T   (transposed output layout)
       kpT [D,S] likewise
       vp  [S,D] = (v^T).T @ Wv^T     (natural layout), scattered into
         v_aug [S, H*(DH+1)]: per head 64 V columns + one ones column.
  2. Attention per (head, q-chunk): scoresT [k,q] tiles = Kh^T.T @ Qh^T
     with k on partitions. Softmax over k needs no max subtraction
     (logits are O(+-8), fp32 exp is safe): exp via ACT with the mask
     bias as a per-partition bias and scale=1/sqrt(DH).
  3. AV: outT [DH+1, q] = [Vh | 1].T @ expT accumulated over k tiles;
     row DH is the softmax denominator (free via the ones column).
  4. Transpose outT 128-col blocks -> [q, DH+1] via a REGULAR fp32r
     matmul against an identity (transpose-mode interleave stalls fp32r
     streams ~1.3us/mm); per-partition reciprocal of col DH;
     tensor_scalar multiply -> normalized head output written straight
     into the assembled [128, D] output tile.

fp32r HW quirks found by microbenchmark (honor these):
  - moving dim (output free size) must be EVEN -> identity matmul uses
    N=66, not 65.
  - tile_position / base_partition=64 operands stall ~1.5us per matmul
    (and blocked tile_position streams can hang the device). All score
    matmuls therefore use full K=128 with ZERO-PADDED per-head K tiles:
    head in one 64-row half, zeros in the other; the matching qpT pair
    tile rows are annihilated by the zeros. Uniform base-0 K=128
    streams measure ~232 ns/mm.
"""

import os
import sys

for _p in (
    "/root/.axon_site",
    "/root/.axon_site/_ro/trn_rl_repo",
    "/root/.axon_site/_ro/pypackages",
    "/opt/trn_rl_repo",
):
    if os.path.isdir(_p) and _p not in sys.path:
        sys.path.append(_p)

import numpy as np

import concourse.bass as bass
import concourse.tile as tile
import concourse.mybir as mybir
from concourse import bacc
from concourse.bass_utils import run_bass_kernel_spmd
from concourse.masks import make_identity

B, S, D, H = 8, 1024, 1024, 16
DH = D // H  # 64
N_CORES = 8
P = 128  # partitions

F32 = mybir.dt.float32
F32R = mybir.dt.float32r


def build_bass(s=S, d=D, h=H, debug=False):
    """Build the per-core Bass program. Same program on all 8 cores."""
    dh = d // h
    kt_n = d // P          # contraction tiles (projections)
    ot_n = d // P          # output-feature tiles
    st_n = s // P          # sequence tiles of 128
    ch = 512 if s % 512 == 0 else s   # moving-dim chunk (<= 512, fp32 PSUM bank)
    ch_n = s // ch         # chunks per sequence
    qb_n = ch // P         # 128-q blocks per chunk
    hp_n = P // dh         # heads per 128-partition tile (2)
    vaug_w = h * (dh + 1)  # v_aug width

    nc = bacc.Bacc(
        "TRN2", target_bir_lowering=False, debug=debug, num_devices=N_CORES
    )

    # big operands are declared float32r in DRAM (same bytes as the fp32
    # host arrays) so plain HWDGE DMAs feed the fp32r matmuls directly
    qT = nc.dram_tensor("qT", (d, s), F32R, kind="ExternalInput").ap()
    kT = nc.dram_tensor("kT", (d, s), F32R, kind="ExternalInput").ap()
    vT = nc.dram_tensor("vT", (d, s), F32R, kind="ExternalInput").ap()
    wqT = nc.dram_tensor("wqT", (d, d), F32R, kind="ExternalInput").ap()
    wkT = nc.dram_tensor("wkT", (d, d), F32R, kind="ExternalInput").ap()
    wvT = nc.dram_tensor("wvT", (d, d), F32R, kind="ExternalInput").ap()
    bqT = nc.dram_tensor("bqT", (P, ot_n), F32, kind="ExternalInput").ap()
    bkT = nc.dram_tensor("bkT", (P, ot_n), F32, kind="ExternalInput").ap()
    # per head: [bv head-slice (dh) | 1.0] — the trailing 1.0 seeds the
    # ones column of v_aug (softmax denominator trick)
    bvB = nc.dram_tensor("bvB", (P, h * (d // h + 1)), F32, kind="ExternalInput").ap()
    mb = nc.dram_tensor("mb", (P, st_n), F32, kind="ExternalInput").ap()
    outd = nc.dram_tensor("out", (s, d), F32, kind="ExternalOutput").ap()

    with tile.TileContext(nc) as tc:
        with tc.tile_pool(name="singles", bufs=1) as singles:
            ident = singles.tile([P, P], F32)
            make_identity(nc, ident)
            # fp32r copy of the identity for the transpose matmuls
            idr = singles.tile([P, P], F32R)
            nc.vector.tensor_copy(idr, ident)
            mb_t = singles.tile([P, st_n], F32)
            nc.sync.dma_start(out=mb_t, in_=mb)
            bq_t = singles.tile([P, ot_n], F32)
            nc.sync.dma_start(out=bq_t, in_=bqT)
            bk_t = singles.tile([P, ot_n], F32)
            nc.sync.dma_start(out=bk_t, in_=bkT)
            bv_t = singles.tile([P, vaug_w], F32)
            nc.sync.dma_start(out=bv_t, in_=bvB)

            # ---- persistent phase-A outputs ----
            # qp: ot_n pair-tiles [128, s]; kp: h zero-padded head tiles
            with tc.tile_pool(name="projout", bufs=ot_n + h) as projout, \
                 tc.tile_pool(name="vaug", bufs=st_n) as vaugp:
                qp_tiles = []
                kp_tiles = []
                vaug_tiles = []

                # ================= Phase A: projections =================
                # operands load in [P, cw] column-chunks so a chunk's slots
                # free as soon as its last consumer retires -> the next
                # projection's DMA prefetches during the current one
                cw = min(512, d)
                cn = d // cw  # chunks per kt-row of a weight
                with tc.tile_pool(name="wpool", bufs=kt_n * cn + 2) as wpool, \
                     tc.tile_pool(name="inpool", bufs=kt_n * (s // cw) + 1) as inpool, \
                     tc.tile_pool(name="ppsum", bufs=6, space="PSUM") as ppsum:

                    def load_chunks(pool, dram, tag, interleave_with=None):
                        """chunks[kt][c] = [P, cw] slice of dram rows kt*P.
                        DMAs issue column-chunk-major (all kt of chunk 0
                        first) so the first consumer group's operands land
                        first; optionally interleaved with a second
                        (pool, dram, tag) spec at chunk granularity."""
                        specs = [(pool, dram, tag)]
                        if interleave_with is not None:
                            specs.append(interleave_with)
                        out = []
                        for pl, dr, tg in specs:
                            ncols = dr.shape[1]
                            out.append([[pl.tile([P, cw], F32R, tag=tg,
                                                 name=f"{tg}_{kt}_{c}")
                                         for c in range(ncols // cw)]
                                        for kt in range(kt_n)])
                        maxc = max(len(t[0]) for t in out)
                        for c in range(maxc):
                            for (pl, dr, tg), tiles in zip(specs, out):
                                if c >= len(tiles[0]):
                                    continue
                                for kt in range(kt_n):
                                    nc.sync.dma_start(
                                        out=tiles[kt][c],
                                        in_=dr[kt * P:(kt + 1) * P,
                                               c * cw:(c + 1) * cw],
                                    )
                        return out if interleave_with is not None else out[0]

                    def wslice(tiles, kt, col0, width):
                        c, off = divmod(col0, cw)
                        assert off + width <= cw
                        return tiles[kt][c][:, off:off + width]

                    # --- qpT (pair tiles, transposed-output projection) ---
                    w_tiles, x_tiles = load_chunks(
                        wpool, wqT, "w", interleave_with=(inpool, qT, "in"))
                    for ot in range(ot_n):
                        po = projout.tile([P, s], F32R, tag="projout",
                                          name=f"qp_{ot}")
                        qp_tiles.append(po)
                        for sc in range(ch_n):
                            ps = ppsum.tile([P, ch], F32, tag="ppsum")
                            for kt in range(kt_n):
                                nc.tensor.matmul(
                                    ps,
                                    wslice(w_tiles, kt, ot * P, P),
                                    wslice(x_tiles, kt, sc * ch, ch),
                                    start=(kt == 0),
                                    stop=(kt == kt_n - 1),
                                )
                            nc.vector.tensor_scalar_add(
                                po[:, sc * ch:(sc + 1) * ch],
                                ps,
                                bq_t[:, ot:ot + 1],
                            )

                    # --- kpT (per-head zero-padded tiles) ---
                    w_tiles, x_tiles = load_chunks(
                        wpool, wkT, "w", interleave_with=(inpool, kT, "in"))
                    for ot in range(ot_n):
                        heads = []
                        for hp in range(hp_n):
                            kpo = projout.tile([P, s], F32R, tag="projout",
                                               name=f"kp_{ot}_{hp}")
                            kp_tiles.append(kpo)
                            heads.append(kpo)
                            if hp_n > 1:
                                # zero the unused 64-row half: DVE multiply
                                # of an (already final) qp tile by 0.0
                                pad0 = 0 if hp else dh
                                nc.vector.tensor_scalar_mul(
                                    kpo[pad0:pad0 + (P - dh), :],
                                    qp_tiles[ot][pad0:pad0 + (P - dh), :],
                                    0.0,
                                )
                        for sc in range(ch_n):
                            ps = ppsum.tile([P, ch], F32, tag="ppsum")
                            for kt in range(kt_n):
                                nc.tensor.matmul(
                                    ps,
                                    wslice(w_tiles, kt, ot * P, P),
                                    wslice(x_tiles, kt, sc * ch, ch),
                                    start=(kt == 0),
                                    stop=(kt == kt_n - 1),
                                )
                            for hp in range(hp_n):
                                rows = slice(hp * dh, (hp + 1) * dh)
                                nc.vector.tensor_scalar_add(
                                    heads[hp][rows, sc * ch:(sc + 1) * ch],
                                    ps[rows, :],
                                    bk_t[rows, ot:ot + 1],
                                )

                    # --- vp -> v_aug (natural layout + ones columns) ---
                    x_tiles, w_tiles = load_chunks(
                        inpool, vT, "in", interleave_with=(wpool, wvT, "w"))
                    oc_n = d // ch
                    bv_g = bv_t.rearrange("p (g c) -> p g c", c=dh + 1)
                    for st in range(st_n):
                        va = vaugp.tile([P, vaug_w], F32R, tag="vaug")
                        vaug_tiles.append(va)
                        va_g = va.rearrange("p (g c) -> p g c", c=dh + 1)
                        for oc in range(oc_n):
                            ps = ppsum.tile([P, ch], F32, tag="ppsum")
                            for kt in range(kt_n):
                                nc.tensor.matmul(
                                    ps,
                                    wslice(x_tiles, kt, st * P, P),
                                    wslice(w_tiles, kt, oc * ch, ch),
                                    start=(kt == 0),
                                    stop=(kt == kt_n - 1),
                                )
                            g0 = oc * (ch // dh)  # first head group in chunk
                            gn = ch // dh
                            nc.vector.tensor_tensor(
                                out=va_g[:, g0:g0 + gn, 0:dh],
                                in0=ps.rearrange("p (g c) -> p g c", c=dh),
                                in1=bv_g[:, g0:g0 + gn, 0:dh],
                                op=mybir.AluOpType.add,
                            )
                        nc.vector.tensor_copy(
                            va_g[:, :, dh:dh + 1], bv_g[:, :, dh:dh + 1]
                        )

                # ================= Phase B: attention =================
                with tc.tile_pool(name="expp", bufs=2 * st_n) as expp, \
                     tc.tile_pool(name="otsp", bufs=4) as otsp, \
                     tc.tile_pool(name="finalp", bufs=qb_n + 2) as finalp, \
                     tc.tile_pool(name="rcpp", bufs=8) as rcpp, \
                     tc.tile_pool(name="spsum", bufs=4, space="PSUM") as spsum, \
                     tc.tile_pool(name="opsum", bufs=2, space="PSUM") as opsum, \
                     tc.tile_pool(name="tpsum", bufs=2, space="PSUM") as tpsum:

                    for qc in range(ch_n):  # q chunk of `ch` columns
                        finals = []
                        for qb in range(qb_n):
                            fin = finalp.tile([P, d], F32, tag="final", name=f"fin_{qc}_{qb}")
                            finals.append(fin)
                        for h2 in range(h // hp_n):  # head pairs
                            # scores + exp: full-K128 matmuls against the
                            # zero-padded per-head kp tiles (no tile_position)
                            exp_tiles = {}
                            for hp in range(hp_n):
                                hh = h2 * hp_n + hp
                                for kt in range(st_n):
                                    sc_ps = spsum.tile([P, ch], F32, tag="spsum")
                                    nc.tensor.matmul(
                                        sc_ps,
                                        kp_tiles[hh][:, kt * P:(kt + 1) * P],
                                        qp_tiles[h2][:, qc * ch:(qc + 1) * ch],
                                        start=True,
                                        stop=True,
                                    )
                                    et = expp.tile([P, ch], F32R, tag="exp")
                                    nc.scalar.activation(
                                        et,
                                        sc_ps,
                                        mybir.ActivationFunctionType.Exp,
                                        bias=mb_t[:, kt:kt + 1],
                                        scale=1.0 / float(np.sqrt(dh)),
                                    )
                                    exp_tiles[(hh, kt)] = et
                            # AV + normalize per head of the pair
                            for hp in range(hp_n):
                                hh = h2 * hp_n + hp
                                ot_ps = opsum.tile([dh + 1, ch], F32, tag="opsum")
                                for kt in range(st_n):
                                    nc.tensor.matmul(
                                        ot_ps,
                                        vaug_tiles[kt][
                                            :, hh * (dh + 1):(hh + 1) * (dh + 1)
                                        ],
                                        exp_tiles[(hh, kt)],
                                        start=(kt == 0),
                                        stop=(kt == st_n - 1),
                                    )
                                ots = otsp.tile([dh + 1, ch], F32R, tag="ots")
                                nc.vector.tensor_copy(ots, ot_ps)
                                for qb in range(qb_n):
                                    # transpose via REGULAR fp32r matmul with
                                    # identity; fp32r needs even N -> dh+2
                                    tr = tpsum.tile([P, dh + 2], F32, tag="tpsum")
                                    nc.tensor.matmul(
                                        tr,
                                        ots[:, qb * P:(qb + 1) * P],
                                        idr[0:dh + 1, 0:dh + 2],
                                        start=True,
                                        stop=True,
                                    )
                                    rcp = rcpp.tile([P, 1], F32, tag="rcp")
                                    nc.vector.reciprocal(rcp, tr[:, dh:dh + 1])
                                    nc.vector.tensor_scalar_mul(
                                        finals[qb][:, hh * dh:(hh + 1) * dh],
                                        tr[:, 0:dh],
                                        rcp,
                                    )
                        for qb in range(qb_n):
                            row0 = qc * ch + qb * P
                            nc.sync.dma_start(
                                out=outd[row0:row0 + P, :], in_=finals[qb]
                            )

    return nc


_CACHE = {}


def _get_compiled():
    if "nc" not in _CACHE:
        nc = build_bass()
        nc.compile()
        _CACHE["nc"] = nc
    return _CACHE["nc"]


def kernel(q, k, v, mask, Wq, bq, Wk, bk, Wv, bv):
    q = np.asarray(q, dtype=np.float32)
    k = np.asarray(k, dtype=np.float32)
    v = np.asarray(v, dtype=np.float32)
    mask = np.asarray(mask, dtype=np.float32)
    Wq = np.asarray(Wq, dtype=np.float32)
    Wk = np.asarray(Wk, dtype=np.float32)
    Wv = np.asarray(Wv, dtype=np.float32)
    bq = np.asarray(bq, dtype=np.float32)
    bk = np.asarray(bk, dtype=np.float32)
    bv = np.asarray(bv, dtype=np.float32)

    nc = _get_compiled()

    ot_n = D // P
    st_n = S // P
    # shared (per-core identical) host-side layout prep
    wqT = np.ascontiguousarray(Wq.T)
    wkT = np.ascontiguousarray(Wk.T)
    wvT = np.ascontiguousarray(Wv.T)
    bqT = np.ascontiguousarray(bq.reshape(ot_n, P).T)
    bkT = np.ascontiguousarray(bk.reshape(ot_n, P).T)
    # [bv head-slice | 1.0] per head, broadcast across partitions
    bv_aug = np.concatenate(
        [np.concatenate([bv.reshape(H, DH), np.ones((H, 1), np.float32)], axis=1).reshape(-1)]
    ).astype(np.float32)
    bvB = np.ascontiguousarray(np.broadcast_to(bv_aug, (P, H * (DH + 1))))

    in_maps = []
    for b in range(B):
        mbias = (-10000.0 * (1.0 - mask[b])).astype(np.float32)
        in_maps.append({
            "qT": np.ascontiguousarray(q[b].T),
            "kT": np.ascontiguousarray(k[b].T),
            "vT": np.ascontiguousarray(v[b].T),
            "wqT": wqT,
            "wkT": wkT,
            "wvT": wvT,
            "bqT": bqT,
            "bkT": bkT,
            "bvB": bvB,
            "mb": np.ascontiguousarray(mbias.reshape(st_n, P).T),
        })

    _CACHE["in_maps"] = in_maps
    res = run_bass_kernel_spmd(nc, in_maps, core_ids=list(range(N_CORES)))
    out = np.stack([res.results[b]["out"] for b in range(B)], axis=0)
    return out.astype(np.float32)


# revision 27
# speedup vs baseline: 1.7308x; 1.0266x over previous
"""Multi-headed self-attention on 8 Trainium2 NeuronCores (Bass/Tile).

Problem: B=8, S=1024, D=1024, H=16 heads (DH=64), fp32.
    qp = q @ Wq.T + bq ; kp = k @ Wk.T + bk ; vp = v @ Wv.T + bv
    out = softmax(Qh Kh^T / sqrt(DH) + maskbias) Vh   (per head, merged)

Sharding: data-parallel over batch — one batch element per core.

Per-core device algorithm (all matmuls in fp32r = tf32-like, 4x faster
than fp32 on the PE at equal storage):
  1. Projections with the contraction dim on partitions. Host pre-
     transposes inputs and weights, so q^T/k^T/v^T [D,S] and W^T [D,D]
     load as plain contiguous DMAs (cast to fp32r in-flight by SWDGE).
       qpT [D,S] = (Wq^T)^T.T @ q^T   (transposed output layout)
       kpT [D,S] likewise
       vp  [S,D] = (v^T).T @ Wv^T     (natural layout), scattered into
         v_aug [S, H*(DH+1)]: per head 64 V columns + one ones column.
  2. Attention per (head, q-chunk): scoresT [k,q] tiles = Kh^T.T @ Qh^T
     with k on partitions. Softmax over k needs no max subtraction
     (logits are O(+-8), fp32 exp is safe): exp via ACT with the mask
     bias as a per-partition bias and scale=1/sqrt(DH).
  3. AV: outT [DH+1, q] = [Vh | 1].T @ expT accumulated over k tiles;
     row DH is the softmax denominator (free via the ones column).
  4. Transpose outT 128-col blocks -> [q, DH+1] via a REGULAR fp32r
     matmul against an identity (transpose-mode interleave stalls fp32r
     streams ~1.3us/mm); per-partition reciprocal of col DH;
     tensor_scalar multiply -> normalized head output written straight
     into the assembled [128, D] output tile.

fp32r HW quirks found by microbenchmark (honor these):
  - moving dim (output free size) must be EVEN -> identity matmul uses
    N=66, not 65.
  - tile_position / base_partition=64 operands stall ~1.5us per matmul
    (and blocked tile_position streams can hang the device). All score
    matmuls therefore use full K=128 with ZERO-PADDED per-head K tiles:
    head in one 64-row half, zeros in the other; the matching qpT pair
    tile rows are annihilated by the zeros. Uniform base-0 K=128
    streams measure ~232 ns/mm.
"""

import os
import sys

for _p in (
    "/root/.axon_site",
    "/root/.axon_site/_ro/trn_rl_repo",
    "/root/.axon_site/_ro/pypackages",
    "/opt/trn_rl_repo",
):
    if os.path.isdir(_p) and _p not in sys.path:
        sys.path.append(_p)

import numpy as np

import concourse.bass as bass
import concourse.tile as tile
import concourse.mybir as mybir
from concourse import bacc
from concourse.bass_utils import run_bass_kernel_spmd
from concourse.masks import make_identity

B, S, D, H = 8, 1024, 1024, 16
DH = D // H  # 64
N_CORES = 8
P = 128  # partitions

F32 = mybir.dt.float32
F32R = mybir.dt.float32r


def build_bass(s=S, d=D, h=H, masked=True, debug=False):
    """Build the per-core Bass program. Same program on all 8 cores.

    masked=False (mask known all-ones on host): exp needs no per-k-tile
    bias, so score PSUM tiles pair two k-tiles [P, 2*ch] and one ACT
    instruction exps both — halves ACT instruction overhead."""
    dh = d // h
    kt_n = d // P          # contraction tiles (projections)
    ot_n = d // P          # output-feature tiles
    st_n = s // P          # sequence tiles of 128
    ch = 512 if s % 512 == 0 else s   # moving-dim chunk (<= 512, fp32 PSUM bank)
    ch_n = s // ch         # chunks per sequence
    qb_n = ch // P         # 128-q blocks per chunk
    hp_n = P // dh         # heads per 128-partition tile (2)
    vaug_w = h * (dh + 1)  # v_aug width

    nc = bacc.Bacc(
        "TRN2", target_bir_lowering=False, debug=debug, num_devices=N_CORES
    )

    # big operands are declared float32r in DRAM (same bytes as the fp32
    # host arrays) so plain HWDGE DMAs feed the fp32r matmuls directly
    qT = nc.dram_tensor("qT", (d, s), F32R, kind="ExternalInput").ap()
    kT = nc.dram_tensor("kT", (d, s), F32R, kind="ExternalInput").ap()
    vT = nc.dram_tensor("vT", (d, s), F32R, kind="ExternalInput").ap()
    wqT = nc.dram_tensor("wqT", (d, d), F32R, kind="ExternalInput").ap()
    wkT = nc.dram_tensor("wkT", (d, d), F32R, kind="ExternalInput").ap()
    wvT = nc.dram_tensor("wvT", (d, d), F32R, kind="ExternalInput").ap()
    bqT = nc.dram_tensor("bqT", (P, ot_n), F32, kind="ExternalInput").ap()
    bkT = nc.dram_tensor("bkT", (P, ot_n), F32, kind="ExternalInput").ap()
    # per head: [bv head-slice (dh) | 1.0] — the trailing 1.0 seeds the
    # ones column of v_aug (softmax denominator trick)
    bvB = nc.dram_tensor("bvB", (P, h * (d // h + 1)), F32, kind="ExternalInput").ap()
    mb = nc.dram_tensor("mb", (P, st_n), F32, kind="ExternalInput").ap()
    outd = nc.dram_tensor("out", (s, d), F32, kind="ExternalOutput").ap()

    with tile.TileContext(nc) as tc:
        with tc.tile_pool(name="singles", bufs=1) as singles:
            ident = singles.tile([P, P], F32)
            make_identity(nc, ident)
            # fp32r copy of the identity for the transpose matmuls
            idr = singles.tile([P, P], F32R)
            nc.vector.tensor_copy(idr, ident)
            mb_t = singles.tile([P, st_n], F32)
            nc.sync.dma_start(out=mb_t, in_=mb)
            bq_t = singles.tile([P, ot_n], F32)
            nc.sync.dma_start(out=bq_t, in_=bqT)
            bk_t = singles.tile([P, ot_n], F32)
            nc.sync.dma_start(out=bk_t, in_=bkT)
            bv_t = singles.tile([P, vaug_w], F32)
            nc.sync.dma_start(out=bv_t, in_=bvB)

            # ---- persistent phase-A outputs ----
            # qp: ot_n pair-tiles [128, s]; kp: h zero-padded head tiles
            with tc.tile_pool(name="projout", bufs=ot_n + h) as projout, \
                 tc.tile_pool(name="vaug", bufs=st_n) as vaugp:
                qp_tiles = []
                kp_tiles = []
                vaug_tiles = []

                # ================= Phase A: projections =================
                # operands load in [P, cw] column-chunks so a chunk's slots
                # free as soon as its last consumer retires -> the next
                # projection's DMA prefetches during the current one
                cw = min(512, d)
                cn = d // cw  # chunks per kt-row of a weight
                with tc.tile_pool(name="wpool", bufs=kt_n * cn + 2) as wpool, \
                     tc.tile_pool(name="inpool", bufs=kt_n * (s // cw) + 1) as inpool, \
                     tc.tile_pool(name="ppsum", bufs=6, space="PSUM") as ppsum:

                    def load_chunks(pool, dram, tag, interleave_with=None):
                        """chunks[kt][c] = [P, cw] slice of dram rows kt*P.
                        DMAs issue column-chunk-major (all kt of chunk 0
                        first) so the first consumer group's operands land
                        first; optionally interleaved with a second
                        (pool, dram, tag) spec at chunk granularity."""
                        specs = [(pool, dram, tag)]
                        if interleave_with is not None:
                            specs.append(interleave_with)
                        out = []
                        for pl, dr, tg in specs:
                            ncols = dr.shape[1]
                            out.append([[pl.tile([P, cw], F32R, tag=tg,
                                                 name=f"{tg}_{kt}_{c}")
                                         for c in range(ncols // cw)]
                                        for kt in range(kt_n)])
                        maxc = max(len(t[0]) for t in out)
                        for c in range(maxc):
                            for kt in range(kt_n):
                                for (pl, dr, tg), tiles in zip(specs, out):
                                    if c >= len(tiles[0]):
                                        continue
                                    nc.sync.dma_start(
                                        out=tiles[kt][c],
                                        in_=dr[kt * P:(kt + 1) * P,
                                               c * cw:(c + 1) * cw],
                                    )
                        return out if interleave_with is not None else out[0]

                    def wslice(tiles, kt, col0, width):
                        c, off = divmod(col0, cw)
                        assert off + width <= cw
                        return tiles[kt][c][:, off:off + width]

                    # --- qpT (pair tiles, transposed-output projection) ---
                    w_tiles, x_tiles = load_chunks(
                        wpool, wqT, "w", interleave_with=(inpool, qT, "in"))
                    for ot in range(ot_n):
                        po = projout.tile([P, s], F32R, tag="projout",
                                          name=f"qp_{ot}")
                        qp_tiles.append(po)
                        for sc in range(ch_n):
                            ps = ppsum.tile([P, ch], F32, tag="ppsum")
                            for kt in range(kt_n):
                                nc.tensor.matmul(
                                    ps,
                                    wslice(w_tiles, kt, ot * P, P),
                                    wslice(x_tiles, kt, sc * ch, ch),
                                    start=(kt == 0),
                                    stop=(kt == kt_n - 1),
                                )
                            nc.vector.tensor_scalar_add(
                                po[:, sc * ch:(sc + 1) * ch],
                                ps,
                                bq_t[:, ot:ot + 1],
                            )

                    # --- kpT (per-head zero-padded tiles) ---
                    w_tiles, x_tiles = load_chunks(
                        wpool, wkT, "w", interleave_with=(inpool, kT, "in"))
                    for ot in range(ot_n):
                        heads = []
                        for hp in range(hp_n):
                            kpo = projout.tile([P, s], F32R, tag="projout",
                                               name=f"kp_{ot}_{hp}")
                            kp_tiles.append(kpo)
                            heads.append(kpo)
                            if hp_n > 1:
                                # zero the unused 64-row half: DVE multiply
                                # of an (already final) qp tile by 0.0
                                pad0 = 0 if hp else dh
                                nc.vector.tensor_scalar_mul(
                                    kpo[pad0:pad0 + (P - dh), :],
                                    qp_tiles[ot][pad0:pad0 + (P - dh), :],
                                    0.0,
                                )
                        for sc in range(ch_n):
                            ps = ppsum.tile([P, ch], F32, tag="ppsum")
                            for kt in range(kt_n):
                                nc.tensor.matmul(
                                    ps,
                                    wslice(w_tiles, kt, ot * P, P),
                                    wslice(x_tiles, kt, sc * ch, ch),
                                    start=(kt == 0),
                                    stop=(kt == kt_n - 1),
                                )
                            for hp in range(hp_n):
                                rows = slice(hp * dh, (hp + 1) * dh)
                                nc.vector.tensor_scalar_add(
                                    heads[hp][rows, sc * ch:(sc + 1) * ch],
                                    ps[rows, :],
                                    bk_t[rows, ot:ot + 1],
                                )

                    # --- vp -> v_aug (natural layout + ones columns) ---
                    x_tiles, w_tiles = load_chunks(
                        inpool, vT, "in", interleave_with=(wpool, wvT, "w"))
                    oc_n = d // ch
                    bv_g = bv_t.rearrange("p (g c) -> p g c", c=dh + 1)
                    for st in range(st_n):
                        va = vaugp.tile([P, vaug_w], F32R, tag="vaug")
                        vaug_tiles.append(va)
                        va_g = va.rearrange("p (g c) -> p g c", c=dh + 1)
                        for oc in range(oc_n):
                            ps = ppsum.tile([P, ch], F32, tag="ppsum")
                            for kt in range(kt_n):
                                nc.tensor.matmul(
                                    ps,
                                    wslice(x_tiles, kt, st * P, P),
                                    wslice(w_tiles, kt, oc * ch, ch),
                                    start=(kt == 0),
                                    stop=(kt == kt_n - 1),
                                )
                            g0 = oc * (ch // dh)  # first head group in chunk
                            gn = ch // dh
                            nc.vector.tensor_tensor(
                                out=va_g[:, g0:g0 + gn, 0:dh],
                                in0=ps.rearrange("p (g c) -> p g c", c=dh),
                                in1=bv_g[:, g0:g0 + gn, 0:dh],
                                op=mybir.AluOpType.add,
                            )
                        nc.vector.tensor_copy(
                            va_g[:, :, dh:dh + 1], bv_g[:, :, dh:dh + 1]
                        )

                # ================= Phase B: attention =================
                # unmasked: score psums pair two k-tiles -> exp spans both
                kt_pair = 1 if masked else min(2, st_n)
                sp_bufs = 4 if masked else 3
                with tc.tile_pool(name="expp", bufs=2 * st_n // kt_pair) as expp, \
                     tc.tile_pool(name="otsp", bufs=4) as otsp, \
                     tc.tile_pool(name="finalp", bufs=qb_n + 2) as finalp, \
                     tc.tile_pool(name="rcpp", bufs=8) as rcpp, \
                     tc.tile_pool(name="spsum", bufs=sp_bufs, space="PSUM") as spsum, \
                     tc.tile_pool(name="opsum", bufs=2 if masked else 1,
                                  space="PSUM") as opsum, \
                     tc.tile_pool(name="tpsum", bufs=2 if masked else 1,
                                  space="PSUM") as tpsum:

                    for qc in range(ch_n):  # q chunk of `ch` columns
                        finals = []
                        for qb in range(qb_n):
                            fin = finalp.tile([P, d], F32, tag="final", name=f"fin_{qc}_{qb}")
                            finals.append(fin)
                        for h2 in range(h // hp_n):  # head pairs
                            # scores + exp: full-K128 matmuls against the
                            # zero-padded per-head kp tiles (no tile_position)
                            exp_tiles = {}
                            for hp in range(hp_n):
                                hh = h2 * hp_n + hp
                                for kt2 in range(st_n // kt_pair):
                                    sc_ps = spsum.tile([P, kt_pair * ch], F32,
                                                       tag="spsum")
                                    for j in range(kt_pair):
                                        kt = kt2 * kt_pair + j
                                        nc.tensor.matmul(
                                            sc_ps[:, j * ch:(j + 1) * ch],
                                            kp_tiles[hh][:, kt * P:(kt + 1) * P],
                                            qp_tiles[h2][:, qc * ch:(qc + 1) * ch],
                                            start=True,
                                            stop=True,
                                        )
                                    et = expp.tile([P, kt_pair * ch], F32R,
                                                   tag="exp")
                                    if masked:
                                        nc.scalar.activation(
                                            et,
                                            sc_ps,
                                            mybir.ActivationFunctionType.Exp,
                                            bias=mb_t[:, kt2:kt2 + 1],
                                            scale=1.0 / float(np.sqrt(dh)),
                                        )
                                    else:
                                        nc.scalar.activation(
                                            et,
                                            sc_ps,
                                            mybir.ActivationFunctionType.Exp,
                                            scale=1.0 / float(np.sqrt(dh)),
                                        )
                                    for j in range(kt_pair):
                                        exp_tiles[(hh, kt2 * kt_pair + j)] = \
                                            et[:, j * ch:(j + 1) * ch]
                            # AV + normalize per head of the pair
                            for hp in range(hp_n):
                                hh = h2 * hp_n + hp
                                ot_ps = opsum.tile([dh + 1, ch], F32, tag="opsum")
                                for kt in range(st_n):
                                    nc.tensor.matmul(
                                        ot_ps,
                                        vaug_tiles[kt][
                                            :, hh * (dh + 1):(hh + 1) * (dh + 1)
                                        ],
                                        exp_tiles[(hh, kt)],
                                        start=(kt == 0),
                                        stop=(kt == st_n - 1),
                                    )
                                ots = otsp.tile([dh + 1, ch], F32R, tag="ots")
                                nc.vector.tensor_copy(ots, ot_ps)
                                for qb in range(qb_n):
                                    # transpose via REGULAR fp32r matmul with
                                    # identity; fp32r needs even N -> dh+2
                                    tr = tpsum.tile([P, dh + 2], F32, tag="tpsum")
                                    nc.tensor.matmul(
                                        tr,
                                        ots[:, qb * P:(qb + 1) * P],
                                        idr[0:dh + 1, 0:dh + 2],
                                        start=True,
                                        stop=True,
                                    )
                                    rcp = rcpp.tile([P, 1], F32, tag="rcp")
                                    nc.vector.reciprocal(rcp, tr[:, dh:dh + 1])
                                    nc.vector.tensor_scalar_mul(
                                        finals[qb][:, hh * dh:(hh + 1) * dh],
                                        tr[:, 0:dh],
                                        rcp,
                                    )
                        for qb in range(qb_n):
                            row0 = qc * ch + qb * P
                            nc.sync.dma_start(
                                out=outd[row0:row0 + P, :], in_=finals[qb]
                            )

    return nc


_CACHE = {}


def _get_compiled(masked=False):
    key = ("nc", masked)
    if key not in _CACHE:
        nc = build_bass(masked=masked)
        nc.compile()
        _CACHE[key] = nc
    return _CACHE[key]


def kernel(q, k, v, mask, Wq, bq, Wk, bk, Wv, bv):
    q = np.asarray(q, dtype=np.float32)
    k = np.asarray(k, dtype=np.float32)
    v = np.asarray(v, dtype=np.float32)
    mask = np.asarray(mask, dtype=np.float32)
    Wq = np.asarray(Wq, dtype=np.float32)
    Wk = np.asarray(Wk, dtype=np.float32)
    Wv = np.asarray(Wv, dtype=np.float32)
    bq = np.asarray(bq, dtype=np.float32)
    bk = np.asarray(bk, dtype=np.float32)
    bv = np.asarray(bv, dtype=np.float32)

    masked = not bool(np.all(mask == 1.0))
    nc = _get_compiled(masked=masked)

    ot_n = D // P
    st_n = S // P
    # shared (per-core identical) host-side layout prep
    wqT = np.ascontiguousarray(Wq.T)
    wkT = np.ascontiguousarray(Wk.T)
    wvT = np.ascontiguousarray(Wv.T)
    bqT = np.ascontiguousarray(bq.reshape(ot_n, P).T)
    bkT = np.ascontiguousarray(bk.reshape(ot_n, P).T)
    # [bv head-slice | 1.0] per head, broadcast across partitions
    bv_aug = np.concatenate(
        [np.concatenate([bv.reshape(H, DH), np.ones((H, 1), np.float32)], axis=1).reshape(-1)]
    ).astype(np.float32)
    bvB = np.ascontiguousarray(np.broadcast_to(bv_aug, (P, H * (DH + 1))))

    in_maps = []
    for b in range(B):
        mbias = (-10000.0 * (1.0 - mask[b])).astype(np.float32)
        in_maps.append({
            "qT": np.ascontiguousarray(q[b].T),
            "kT": np.ascontiguousarray(k[b].T),
            "vT": np.ascontiguousarray(v[b].T),
            "wqT": wqT,
            "wkT": wkT,
            "wvT": wvT,
            "bqT": bqT,
            "bkT": bkT,
            "bvB": bvB,
            "mb": np.ascontiguousarray(mbias.reshape(st_n, P).T),
        })

    _CACHE["in_maps"] = in_maps
    res = run_bass_kernel_spmd(nc, in_maps, core_ids=list(range(N_CORES)))
    out = np.stack([res.results[b]["out"] for b in range(B)], axis=0)
    return out.astype(np.float32)


# revision 28
# speedup vs baseline: 1.7943x; 1.0367x over previous
"""Multi-headed self-attention on 8 Trainium2 NeuronCores (Bass/Tile).

Problem: B=8, S=1024, D=1024, H=16 heads (DH=64), fp32.
    qp = q @ Wq.T + bq ; kp = k @ Wk.T + bk ; vp = v @ Wv.T + bv
    out = softmax(Qh Kh^T / sqrt(DH) + maskbias) Vh   (per head, merged)

Sharding: data-parallel over batch — one batch element per core.

Per-core device algorithm (all matmuls in fp32r = tf32-like, 4x faster
than fp32 on the PE at equal storage):
  1. Projections with the contraction dim on partitions. Host pre-
     transposes inputs and weights, so q^T/k^T/v^T [D,S] and W^T [D,D]
     load as plain contiguous DMAs (cast to fp32r in-flight by SWDGE).
       qpT [D,S] = (Wq^T)^T.T @ q^T   (transposed output layout)
       kpT [D,S] likewise
       vp  [S,D] = (v^T).T @ Wv^T     (natural layout), scattered into
         v_aug [S, H*(DH+1)]: per head 64 V columns + one ones column.
  2. Attention per (head, q-chunk): scoresT [k,q] tiles = Kh^T.T @ Qh^T
     with k on partitions. Softmax over k needs no max subtraction
     (logits are O(+-8), fp32 exp is safe): exp via ACT with the mask
     bias as a per-partition bias and scale=1/sqrt(DH).
  3. AV: outT [DH+1, q] = [Vh | 1].T @ expT accumulated over k tiles;
     row DH is the softmax denominator (free via the ones column).
  4. Transpose outT 128-col blocks -> [q, DH+1] via a REGULAR fp32r
     matmul against an identity (transpose-mode interleave stalls fp32r
     streams ~1.3us/mm); per-partition reciprocal of col DH;
     tensor_scalar multiply -> normalized head output written straight
     into the assembled [128, D] output tile.

fp32r HW quirks found by microbenchmark (honor these):
  - moving dim (output free size) must be EVEN -> identity matmul uses
    N=66, not 65.
  - tile_position / base_partition=64 operands stall ~1.5us per matmul
    (and blocked tile_position streams can hang the device). All score
    matmuls therefore use full K=128 with ZERO-PADDED per-head K tiles:
    head in one 64-row half, zeros in the other; the matching qpT pair
    tile rows are annihilated by the zeros. Uniform base-0 K=128
    streams measure ~232 ns/mm.
"""

import os
import sys

for _p in (
    "/root/.axon_site",
    "/root/.axon_site/_ro/trn_rl_repo",
    "/root/.axon_site/_ro/pypackages",
    "/opt/trn_rl_repo",
):
    if os.path.isdir(_p) and _p not in sys.path:
        sys.path.append(_p)

import numpy as np

import concourse.bass as bass
import concourse.tile as tile
import concourse.mybir as mybir
from concourse import bacc
from concourse.bass_utils import run_bass_kernel_spmd
from concourse.masks import make_identity

B, S, D, H = 8, 1024, 1024, 16
DH = D // H  # 64
N_CORES = 8
P = 128  # partitions

F32 = mybir.dt.float32
F32R = mybir.dt.float32r


def build_bass(s=S, d=D, h=H, masked=True, debug=False):
    """Build the per-core Bass program. Same program on all 8 cores.

    masked=False (mask known all-ones on host): exp needs no per-k-tile
    bias, so score PSUM tiles pair two k-tiles [P, 2*ch] and one ACT
    instruction exps both — halves ACT instruction overhead."""
    dh = d // h
    kt_n = d // P          # contraction tiles (projections)
    ot_n = d // P          # output-feature tiles
    st_n = s // P          # sequence tiles of 128
    ch = 512 if s % 512 == 0 else s   # moving-dim chunk (<= 512, fp32 PSUM bank)
    ch_n = s // ch         # chunks per sequence
    qb_n = ch // P         # 128-q blocks per chunk
    hp_n = P // dh         # heads per 128-partition tile (2)
    vaug_w = h * (dh + 1)  # v_aug width

    nc = bacc.Bacc(
        "TRN2", target_bir_lowering=False, debug=debug, num_devices=N_CORES
    )

    # big operands are declared float32r in DRAM (same bytes as the fp32
    # host arrays) so plain HWDGE DMAs feed the fp32r matmuls directly
    qT = nc.dram_tensor("qT", (d, s), F32R, kind="ExternalInput").ap()
    kT = nc.dram_tensor("kT", (d, s), F32R, kind="ExternalInput").ap()
    vT = nc.dram_tensor("vT", (d, s), F32R, kind="ExternalInput").ap()
    wqT = nc.dram_tensor("wqT", (d, d), F32R, kind="ExternalInput").ap()
    wkT = nc.dram_tensor("wkT", (d, d), F32R, kind="ExternalInput").ap()
    wvT = nc.dram_tensor("wvT", (d, d), F32R, kind="ExternalInput").ap()
    bqT = nc.dram_tensor("bqT", (P, ot_n), F32, kind="ExternalInput").ap()
    bkT = nc.dram_tensor("bkT", (P, ot_n), F32, kind="ExternalInput").ap()
    # per head: [bv head-slice (dh) | 1.0] — the trailing 1.0 seeds the
    # ones column of v_aug (softmax denominator trick)
    bvB = nc.dram_tensor("bvB", (P, h * (d // h + 1)), F32, kind="ExternalInput").ap()
    mb = nc.dram_tensor("mb", (P, st_n), F32, kind="ExternalInput").ap()
    outd = nc.dram_tensor("out", (s, d), F32, kind="ExternalOutput").ap()

    with tile.TileContext(nc) as tc:
        with tc.tile_pool(name="singles", bufs=1) as singles:
            ident = singles.tile([P, P], F32)
            make_identity(nc, ident)
            # fp32r copy of the identity for the transpose matmuls
            idr = singles.tile([P, P], F32R)
            nc.vector.tensor_copy(idr, ident)
            mb_t = singles.tile([P, st_n], F32)
            nc.sync.dma_start(out=mb_t, in_=mb)
            bq_t = singles.tile([P, ot_n], F32)
            nc.sync.dma_start(out=bq_t, in_=bqT)
            bk_t = singles.tile([P, ot_n], F32)
            nc.sync.dma_start(out=bk_t, in_=bkT)
            bv_t = singles.tile([P, vaug_w], F32)
            nc.sync.dma_start(out=bv_t, in_=bvB)

            # ---- persistent phase-A outputs ----
            # qp: ot_n pair-tiles [128, s]; kp: h zero-padded head tiles
            with tc.tile_pool(name="projout", bufs=ot_n + h) as projout, \
                 tc.tile_pool(name="vaug", bufs=st_n) as vaugp:
                qp_tiles = []
                kp_tiles = []
                vaug_tiles = []

                # ================= Phase A: projections =================
                # operands load in [P, cw] column-chunks so a chunk's slots
                # free as soon as its last consumer retires -> the next
                # projection's DMA prefetches during the current one
                cw = min(512, d)
                cn = d // cw  # chunks per kt-row of a weight
                with tc.tile_pool(name="wpool", bufs=kt_n * cn + 2) as wpool, \
                     tc.tile_pool(name="inpool", bufs=kt_n * (s // cw) + 1) as inpool, \
                     tc.tile_pool(name="ppsum", bufs=6, space="PSUM") as ppsum:

                    def load_chunks(pool, dram, tag, interleave_with=None):
                        """chunks[kt][c] = [P, cw] slice of dram rows kt*P.
                        DMAs issue column-chunk-major (all kt of chunk 0
                        first) so the first consumer group's operands land
                        first; optionally interleaved with a second
                        (pool, dram, tag) spec at chunk granularity."""
                        specs = [(pool, dram, tag)]
                        if interleave_with is not None:
                            specs.append(interleave_with)
                        out = []
                        for pl, dr, tg in specs:
                            ncols = dr.shape[1]
                            out.append([[pl.tile([P, cw], F32R, tag=tg,
                                                 name=f"{tg}_{kt}_{c}")
                                         for c in range(ncols // cw)]
                                        for kt in range(kt_n)])
                        maxc = max(len(t[0]) for t in out)
                        for c in range(maxc):
                            for kt in range(kt_n):
                                for (pl, dr, tg), tiles in zip(specs, out):
                                    if c >= len(tiles[0]):
                                        continue
                                    nc.sync.dma_start(
                                        out=tiles[kt][c],
                                        in_=dr[kt * P:(kt + 1) * P,
                                               c * cw:(c + 1) * cw],
                                    )
                        return out if interleave_with is not None else out[0]

                    def wslice(tiles, kt, col0, width):
                        c, off = divmod(col0, cw)
                        assert off + width <= cw
                        return tiles[kt][c][:, off:off + width]

                    # --- qpT (pair tiles, transposed-output projection) ---
                    w_tiles, x_tiles = load_chunks(
                        wpool, wqT, "w", interleave_with=(inpool, qT, "in"))
                    for ot in range(ot_n):
                        po = projout.tile([P, s], F32R, tag="projout",
                                          name=f"qp_{ot}")
                        qp_tiles.append(po)
                        for sc in range(ch_n):
                            ps = ppsum.tile([P, ch], F32, tag="ppsum")
                            for kt in range(kt_n):
                                nc.tensor.matmul(
                                    ps,
                                    wslice(w_tiles, kt, ot * P, P),
                                    wslice(x_tiles, kt, sc * ch, ch),
                                    start=(kt == 0),
                                    stop=(kt == kt_n - 1),
                                )
                            nc.vector.tensor_scalar_add(
                                po[:, sc * ch:(sc + 1) * ch],
                                ps,
                                bq_t[:, ot:ot + 1],
                            )

                    # --- kpT (per-head zero-padded tiles) ---
                    w_tiles, x_tiles = load_chunks(
                        wpool, wkT, "w", interleave_with=(inpool, kT, "in"))
                    for ot in range(ot_n):
                        heads = []
                        for hp in range(hp_n):
                            kpo = projout.tile([P, s], F32R, tag="projout",
                                               name=f"kp_{ot}_{hp}")
                            kp_tiles.append(kpo)
                            heads.append(kpo)
                            if hp_n > 1:
                                # zero the unused 64-row half: DVE multiply
                                # of an (already final) qp tile by 0.0
                                pad0 = 0 if hp else dh
                                nc.vector.tensor_scalar_mul(
                                    kpo[pad0:pad0 + (P - dh), :],
                                    qp_tiles[ot][pad0:pad0 + (P - dh), :],
                                    0.0,
                                )
                        for sc in range(ch_n):
                            ps = ppsum.tile([P, ch], F32, tag="ppsum")
                            for kt in range(kt_n):
                                nc.tensor.matmul(
                                    ps,
                                    wslice(w_tiles, kt, ot * P, P),
                                    wslice(x_tiles, kt, sc * ch, ch),
                                    start=(kt == 0),
                                    stop=(kt == kt_n - 1),
                                )
                            for hp in range(hp_n):
                                rows = slice(hp * dh, (hp + 1) * dh)
                                nc.vector.tensor_scalar_add(
                                    heads[hp][rows, sc * ch:(sc + 1) * ch],
                                    ps[rows, :],
                                    bk_t[rows, ot:ot + 1],
                                )

                    # --- vp -> v_aug (natural layout + ones columns) ---
                    x_tiles, w_tiles = load_chunks(
                        inpool, vT, "in", interleave_with=(wpool, wvT, "w"))
                    oc_n = d // ch
                    bv_g = bv_t.rearrange("p (g c) -> p g c", c=dh + 1)
                    for st in range(st_n):
                        va = vaugp.tile([P, vaug_w], F32R, tag="vaug")
                        vaug_tiles.append(va)
                        va_g = va.rearrange("p (g c) -> p g c", c=dh + 1)
                        for oc in range(oc_n):
                            ps = ppsum.tile([P, ch], F32, tag="ppsum")
                            for kt in range(kt_n):
                                nc.tensor.matmul(
                                    ps,
                                    wslice(x_tiles, kt, st * P, P),
                                    wslice(w_tiles, kt, oc * ch, ch),
                                    start=(kt == 0),
                                    stop=(kt == kt_n - 1),
                                )
                            g0 = oc * (ch // dh)  # first head group in chunk
                            gn = ch // dh
                            nc.vector.tensor_tensor(
                                out=va_g[:, g0:g0 + gn, 0:dh],
                                in0=ps.rearrange("p (g c) -> p g c", c=dh),
                                in1=bv_g[:, g0:g0 + gn, 0:dh],
                                op=mybir.AluOpType.add,
                            )
                        nc.vector.tensor_copy(
                            va_g[:, :, dh:dh + 1], bv_g[:, :, dh:dh + 1]
                        )

                # ================= Phase B: attention =================
                # unmasked: score psums pair two k-tiles -> exp spans both
                kt_pair = 1 if masked else min(2, st_n)
                sp_bufs = 4 if masked else 2
                with tc.tile_pool(name="expp", bufs=2 * st_n // kt_pair) as expp, \
                     tc.tile_pool(name="otsp", bufs=4) as otsp, \
                     tc.tile_pool(name="finalp", bufs=qb_n + 3) as finalp, \
                     tc.tile_pool(name="rcpp", bufs=8) as rcpp, \
                     tc.tile_pool(name="spsum", bufs=sp_bufs, space="PSUM") as spsum, \
                     tc.tile_pool(name="opsum", bufs=2 if masked else 1,
                                  space="PSUM") as opsum, \
                     tc.tile_pool(name="tpsum", bufs=2 if masked else 3,
                                  space="PSUM") as tpsum:

                    for qc in range(ch_n):  # q chunk of `ch` columns
                        finals = []
                        for qb in range(qb_n):
                            fin = finalp.tile([P, d], F32, tag="final", name=f"fin_{qc}_{qb}")
                            finals.append(fin)
                        for h2 in range(h // hp_n):  # head pairs
                            # scores + exp: full-K128 matmuls against the
                            # zero-padded per-head kp tiles (no tile_position)
                            exp_tiles = {}
                            for hp in range(hp_n):
                                hh = h2 * hp_n + hp
                                for kt2 in range(st_n // kt_pair):
                                    sc_ps = spsum.tile([P, kt_pair * ch], F32,
                                                       tag="spsum")
                                    for j in range(kt_pair):
                                        kt = kt2 * kt_pair + j
                                        nc.tensor.matmul(
                                            sc_ps[:, j * ch:(j + 1) * ch],
                                            kp_tiles[hh][:, kt * P:(kt + 1) * P],
                                            qp_tiles[h2][:, qc * ch:(qc + 1) * ch],
                                            start=True,
                                            stop=True,
                                        )
                                    et = expp.tile([P, kt_pair * ch], F32R,
                                                   tag="exp")
                                    if masked:
                                        nc.scalar.activation(
                                            et,
                                            sc_ps,
                                            mybir.ActivationFunctionType.Exp,
                                            bias=mb_t[:, kt2:kt2 + 1],
                                            scale=1.0 / float(np.sqrt(dh)),
                                        )
                                    else:
                                        nc.scalar.activation(
                                            et,
                                            sc_ps,
                                            mybir.ActivationFunctionType.Exp,
                                            scale=1.0 / float(np.sqrt(dh)),
                                        )
                                    for j in range(kt_pair):
                                        exp_tiles[(hh, kt2 * kt_pair + j)] = \
                                            et[:, j * ch:(j + 1) * ch]
                            # AV + normalize per head of the pair
                            for hp in range(hp_n):
                                hh = h2 * hp_n + hp
                                ot_ps = opsum.tile([dh + 1, ch], F32, tag="opsum")
                                for kt in range(st_n):
                                    nc.tensor.matmul(
                                        ot_ps,
                                        vaug_tiles[kt][
                                            :, hh * (dh + 1):(hh + 1) * (dh + 1)
                                        ],
                                        exp_tiles[(hh, kt)],
                                        start=(kt == 0),
                                        stop=(kt == st_n - 1),
                                    )
                                ots = otsp.tile([dh + 1, ch], F32R, tag="ots")
                                hc = ch // 2
                                nc.vector.tensor_copy(ots[:, 0:hc],
                                                      ot_ps[:, 0:hc])
                                nc.vector.tensor_copy(ots[:, hc:ch],
                                                      ot_ps[:, hc:ch])
                                for qb in range(qb_n):
                                    # transpose via REGULAR fp32r matmul with
                                    # identity; fp32r needs even N -> dh+2
                                    tr = tpsum.tile([P, dh + 2], F32, tag="tpsum")
                                    nc.tensor.matmul(
                                        tr,
                                        ots[:, qb * P:(qb + 1) * P],
                                        idr[0:dh + 1, 0:dh + 2],
                                        start=True,
                                        stop=True,
                                    )
                                    rcp = rcpp.tile([P, 1], F32, tag="rcp")
                                    nc.vector.reciprocal(rcp, tr[:, dh:dh + 1])
                                    nc.vector.tensor_scalar_mul(
                                        finals[qb][:, hh * dh:(hh + 1) * dh],
                                        tr[:, 0:dh],
                                        rcp,
                                    )
                        for qb in range(qb_n):
                            row0 = qc * ch + qb * P
                            nc.sync.dma_start(
                                out=outd[row0:row0 + P, :], in_=finals[qb]
                            )

    return nc


_CACHE = {}


def _get_compiled(masked=False):
    key = ("nc", masked)
    if key not in _CACHE:
        nc = build_bass(masked=masked)
        nc.compile()
        _CACHE[key] = nc
    return _CACHE[key]


def kernel(q, k, v, mask, Wq, bq, Wk, bk, Wv, bv):
    q = np.asarray(q, dtype=np.float32)
    k = np.asarray(k, dtype=np.float32)
    v = np.asarray(v, dtype=np.float32)
    mask = np.asarray(mask, dtype=np.float32)
    Wq = np.asarray(Wq, dtype=np.float32)
    Wk = np.asarray(Wk, dtype=np.float32)
    Wv = np.asarray(Wv, dtype=np.float32)
    bq = np.asarray(bq, dtype=np.float32)
    bk = np.asarray(bk, dtype=np.float32)
    bv = np.asarray(bv, dtype=np.float32)

    masked = not bool(np.all(mask == 1.0))
    nc = _get_compiled(masked=masked)

    ot_n = D // P
    st_n = S // P
    # shared (per-core identical) host-side layout prep
    wqT = np.ascontiguousarray(Wq.T)
    wkT = np.ascontiguousarray(Wk.T)
    wvT = np.ascontiguousarray(Wv.T)
    bqT = np.ascontiguousarray(bq.reshape(ot_n, P).T)
    bkT = np.ascontiguousarray(bk.reshape(ot_n, P).T)
    # [bv head-slice | 1.0] per head, broadcast across partitions
    bv_aug = np.concatenate(
        [np.concatenate([bv.reshape(H, DH), np.ones((H, 1), np.float32)], axis=1).reshape(-1)]
    ).astype(np.float32)
    bvB = np.ascontiguousarray(np.broadcast_to(bv_aug, (P, H * (DH + 1))))

    in_maps = []
    for b in range(B):
        mbias = (-10000.0 * (1.0 - mask[b])).astype(np.float32)
        in_maps.append({
            "qT": np.ascontiguousarray(q[b].T),
            "kT": np.ascontiguousarray(k[b].T),
            "vT": np.ascontiguousarray(v[b].T),
            "wqT": wqT,
            "wkT": wkT,
            "wvT": wvT,
            "bqT": bqT,
            "bkT": bkT,
            "bvB": bvB,
            "mb": np.ascontiguousarray(mbias.reshape(st_n, P).T),
        })

    _CACHE["in_maps"] = in_maps
    res = run_bass_kernel_spmd(nc, in_maps, core_ids=list(range(N_CORES)))
    out = np.stack([res.results[b]["out"] for b in range(B)], axis=0)
    return out.astype(np.float32)


# revision 29
# speedup vs baseline: 1.8486x; 1.0303x over previous
"""Multi-headed self-attention on 8 Trainium2 NeuronCores (Bass/Tile).

Problem: B=8, S=1024, D=1024, H=16 heads (DH=64), fp32.
    qp = q @ Wq.T + bq ; kp = k @ Wk.T + bk ; vp = v @ Wv.T + bv
    out = softmax(Qh Kh^T / sqrt(DH) + maskbias) Vh   (per head, merged)

Sharding: data-parallel over batch — one batch element per core.

Per-core device algorithm (all matmuls in fp32r = tf32-like, 4x faster
than fp32 on the PE at equal storage):
  1. Projections with the contraction dim on partitions. Host pre-
     transposes inputs and weights, so q^T/k^T/v^T [D,S] and W^T [D,D]
     load as plain contiguous DMAs (cast to fp32r in-flight by SWDGE).
       qpT [D,S] = (Wq^T)^T.T @ q^T   (transposed output layout)
       kpT [D,S] likewise
       vp  [S,D] = (v^T).T @ Wv^T     (natural layout), scattered into
         v_aug [S, H*(DH+1)]: per head 64 V columns + one ones column.
  2. Attention per (head, q-chunk): scoresT [k,q] tiles = Kh^T.T @ Qh^T
     with k on partitions. Softmax over k needs no max subtraction
     (logits are O(+-8), fp32 exp is safe): exp via ACT with the mask
     bias as a per-partition bias and scale=1/sqrt(DH).
  3. AV: outT [DH+1, q] = [Vh | 1].T @ expT accumulated over k tiles;
     row DH is the softmax denominator (free via the ones column).
  4. Transpose outT 128-col blocks -> [q, DH+1] via a REGULAR fp32r
     matmul against an identity (transpose-mode interleave stalls fp32r
     streams ~1.3us/mm); per-partition reciprocal of col DH;
     tensor_scalar multiply -> normalized head output written straight
     into the assembled [128, D] output tile.

fp32r HW quirks found by microbenchmark (honor these):
  - moving dim (output free size) must be EVEN -> identity matmul uses
    N=66, not 65.
  - tile_position / base_partition=64 operands stall ~1.5us per matmul
    (and blocked tile_position streams can hang the device). All score
    matmuls therefore use full K=128 with ZERO-PADDED per-head K tiles:
    head in one 64-row half, zeros in the other; the matching qpT pair
    tile rows are annihilated by the zeros. Uniform base-0 K=128
    streams measure ~232 ns/mm.
"""

import os
import sys

for _p in (
    "/root/.axon_site",
    "/root/.axon_site/_ro/trn_rl_repo",
    "/root/.axon_site/_ro/pypackages",
    "/opt/trn_rl_repo",
):
    if os.path.isdir(_p) and _p not in sys.path:
        sys.path.append(_p)

import numpy as np

import concourse.bass as bass
import concourse.tile as tile
import concourse.mybir as mybir
from concourse import bacc
from concourse.bass_utils import run_bass_kernel_spmd
from concourse.masks import make_identity

B, S, D, H = 8, 1024, 1024, 16
DH = D // H  # 64
N_CORES = 8
P = 128  # partitions

F32 = mybir.dt.float32
F32R = mybir.dt.float32r


def build_bass(s=S, d=D, h=H, masked=True, debug=False):
    """Build the per-core Bass program. Same program on all 8 cores.

    masked=False (mask known all-ones on host): exp needs no per-k-tile
    bias, so score PSUM tiles pair two k-tiles [P, 2*ch] and one ACT
    instruction exps both — halves ACT instruction overhead."""
    dh = d // h
    kt_n = d // P          # contraction tiles (projections)
    ot_n = d // P          # output-feature tiles
    st_n = s // P          # sequence tiles of 128
    ch = 512 if s % 512 == 0 else s   # moving-dim chunk (<= 512, fp32 PSUM bank)
    ch_n = s // ch         # chunks per sequence
    qb_n = ch // P         # 128-q blocks per chunk
    hp_n = P // dh         # heads per 128-partition tile (2)
    vaug_w = h * (dh + 1)  # v_aug width

    nc = bacc.Bacc(
        "TRN2", target_bir_lowering=False, debug=debug, num_devices=N_CORES
    )

    # big operands are declared float32r in DRAM (same bytes as the fp32
    # host arrays) so plain HWDGE DMAs feed the fp32r matmuls directly
    qT = nc.dram_tensor("qT", (d, s), F32R, kind="ExternalInput").ap()
    kT = nc.dram_tensor("kT", (d, s), F32R, kind="ExternalInput").ap()
    vT = nc.dram_tensor("vT", (d, s), F32R, kind="ExternalInput").ap()
    wqT = nc.dram_tensor("wqT", (d, d), F32R, kind="ExternalInput").ap()
    wkT = nc.dram_tensor("wkT", (d, d), F32R, kind="ExternalInput").ap()
    wvT = nc.dram_tensor("wvT", (d, d), F32R, kind="ExternalInput").ap()
    bqT = nc.dram_tensor("bqT", (P, ot_n), F32, kind="ExternalInput").ap()
    bkT = nc.dram_tensor("bkT", (P, ot_n), F32, kind="ExternalInput").ap()
    # per head: [bv head-slice (dh) | 1.0] — the trailing 1.0 seeds the
    # ones column of v_aug (softmax denominator trick)
    bvB = nc.dram_tensor("bvB", (P, h * (d // h + 1)), F32, kind="ExternalInput").ap()
    mb = nc.dram_tensor("mb", (P, st_n), F32, kind="ExternalInput").ap()
    outd = nc.dram_tensor("out", (s, d), F32, kind="ExternalOutput").ap()

    with tile.TileContext(nc) as tc:
        with tc.tile_pool(name="singles", bufs=1) as singles:
            ident = singles.tile([P, P], F32)
            make_identity(nc, ident)
            # fp32r copy of the identity for the transpose matmuls
            idr = singles.tile([P, P], F32R)
            nc.vector.tensor_copy(idr, ident)
            mb_t = singles.tile([P, st_n], F32)
            nc.sync.dma_start(out=mb_t, in_=mb)
            bq_t = singles.tile([P, ot_n], F32)
            nc.sync.dma_start(out=bq_t, in_=bqT)
            bk_t = singles.tile([P, ot_n], F32)
            nc.sync.dma_start(out=bk_t, in_=bkT)
            bv_t = singles.tile([P, vaug_w], F32)
            nc.sync.dma_start(out=bv_t, in_=bvB)

            # ---- persistent phase-A outputs ----
            # qp: ot_n pair-tiles [128, s]; kp: h zero-padded head tiles
            with tc.tile_pool(name="projout", bufs=ot_n + h) as projout, \
                 tc.tile_pool(name="vaug", bufs=st_n) as vaugp:
                qp_tiles = []
                kp_tiles = []
                vaug_tiles = []

                # ================= Phase A: projections =================
                # operands load in [P, cw] column-chunks so a chunk's slots
                # free as soon as its last consumer retires -> the next
                # projection's DMA prefetches during the current one
                cw = min(512, d)
                cn = d // cw  # chunks per kt-row of a weight
                with tc.tile_pool(name="wpool", bufs=kt_n * cn + 2) as wpool, \
                     tc.tile_pool(name="inpool", bufs=kt_n * (s // cw) + 1) as inpool, \
                     tc.tile_pool(name="ppsum", bufs=6, space="PSUM") as ppsum:

                    def load_chunks(pool, dram, tag, interleave_with=None):
                        """chunks[kt][c] = [P, cw] slice of dram rows kt*P.
                        DMAs issue column-chunk-major (all kt of chunk 0
                        first) so the first consumer group's operands land
                        first; optionally interleaved with a second
                        (pool, dram, tag) spec at chunk granularity."""
                        specs = [(pool, dram, tag)]
                        if interleave_with is not None:
                            specs.append(interleave_with)
                        out = []
                        for pl, dr, tg in specs:
                            ncols = dr.shape[1]
                            out.append([[pl.tile([P, cw], F32R, tag=tg,
                                                 name=f"{tg}_{kt}_{c}")
                                         for c in range(ncols // cw)]
                                        for kt in range(kt_n)])
                        maxc = max(len(t[0]) for t in out)
                        for c in range(maxc):
                            for kt in range(kt_n):
                                for (pl, dr, tg), tiles in zip(specs, out):
                                    if c >= len(tiles[0]):
                                        continue
                                    nc.sync.dma_start(
                                        out=tiles[kt][c],
                                        in_=dr[kt * P:(kt + 1) * P,
                                               c * cw:(c + 1) * cw],
                                    )
                        return out if interleave_with is not None else out[0]

                    def wslice(tiles, kt, col0, width):
                        c, off = divmod(col0, cw)
                        assert off + width <= cw
                        return tiles[kt][c][:, off:off + width]

                    # --- qpT (pair tiles, transposed-output projection) ---
                    w_tiles, x_tiles = load_chunks(
                        wpool, wqT, "w", interleave_with=(inpool, qT, "in"))
                    for ot in range(ot_n):
                        po = projout.tile([P, s], F32R, tag="projout",
                                          name=f"qp_{ot}")
                        qp_tiles.append(po)
                        for sc in range(ch_n):
                            ps = ppsum.tile([P, ch], F32, tag="ppsum")
                            for kt in range(kt_n):
                                nc.tensor.matmul(
                                    ps,
                                    wslice(w_tiles, kt, ot * P, P),
                                    wslice(x_tiles, kt, sc * ch, ch),
                                    start=(kt == 0),
                                    stop=(kt == kt_n - 1),
                                )
                            nc.vector.tensor_scalar_add(
                                po[:, sc * ch:(sc + 1) * ch],
                                ps,
                                bq_t[:, ot:ot + 1],
                            )

                    # --- kpT (per-head zero-padded tiles) ---
                    w_tiles, x_tiles = load_chunks(
                        wpool, wkT, "w", interleave_with=(inpool, kT, "in"))
                    for ot in range(ot_n):
                        heads = []
                        for hp in range(hp_n):
                            kpo = projout.tile([P, s], F32R, tag="projout",
                                               name=f"kp_{ot}_{hp}")
                            kp_tiles.append(kpo)
                            heads.append(kpo)
                            if hp_n > 1:
                                # zero the unused 64-row half: DVE multiply
                                # of an (already final) qp tile by 0.0
                                pad0 = 0 if hp else dh
                                nc.vector.tensor_scalar_mul(
                                    kpo[pad0:pad0 + (P - dh), :],
                                    qp_tiles[ot][pad0:pad0 + (P - dh), :],
                                    0.0,
                                )
                        for sc in range(ch_n):
                            ps = ppsum.tile([P, ch], F32, tag="ppsum")
                            for kt in range(kt_n):
                                nc.tensor.matmul(
                                    ps,
                                    wslice(w_tiles, kt, ot * P, P),
                                    wslice(x_tiles, kt, sc * ch, ch),
                                    start=(kt == 0),
                                    stop=(kt == kt_n - 1),
                                )
                            for hp in range(hp_n):
                                rows = slice(hp * dh, (hp + 1) * dh)
                                nc.vector.tensor_scalar_add(
                                    heads[hp][rows, sc * ch:(sc + 1) * ch],
                                    ps[rows, :],
                                    bk_t[rows, ot:ot + 1],
                                )

                    # --- vp -> v_aug (natural layout + ones columns) ---
                    x_tiles, w_tiles = load_chunks(
                        inpool, vT, "in", interleave_with=(wpool, wvT, "w"))
                    oc_n = d // ch
                    bv_g = bv_t.rearrange("p (g c) -> p g c", c=dh + 1)
                    for st in range(st_n):
                        va = vaugp.tile([P, vaug_w], F32R, tag="vaug")
                        vaug_tiles.append(va)
                        va_g = va.rearrange("p (g c) -> p g c", c=dh + 1)
                        for oc in range(oc_n):
                            ps = ppsum.tile([P, ch], F32, tag="ppsum")
                            for kt in range(kt_n):
                                nc.tensor.matmul(
                                    ps,
                                    wslice(x_tiles, kt, st * P, P),
                                    wslice(w_tiles, kt, oc * ch, ch),
                                    start=(kt == 0),
                                    stop=(kt == kt_n - 1),
                                )
                            g0 = oc * (ch // dh)  # first head group in chunk
                            gn = ch // dh
                            nc.vector.tensor_tensor(
                                out=va_g[:, g0:g0 + gn, 0:dh],
                                in0=ps.rearrange("p (g c) -> p g c", c=dh),
                                in1=bv_g[:, g0:g0 + gn, 0:dh],
                                op=mybir.AluOpType.add,
                            )
                        nc.vector.tensor_copy(
                            va_g[:, :, dh:dh + 1], bv_g[:, :, dh:dh + 1]
                        )

                # ================= Phase B: attention =================
                # unmasked: score psums pair two k-tiles -> exp spans both
                kt_pair = 1 if masked else min(2, st_n)
                sp_bufs = 4 if masked else 2
                with tc.tile_pool(name="expp", bufs=2 * st_n // kt_pair) as expp, \
                     tc.tile_pool(name="otsp", bufs=4) as otsp, \
                     tc.tile_pool(name="finalp", bufs=qb_n + 3) as finalp, \
                     tc.tile_pool(name="rcpp", bufs=8) as rcpp, \
                     tc.tile_pool(name="spsum", bufs=sp_bufs, space="PSUM") as spsum, \
                     tc.tile_pool(name="opsum", bufs=2 if masked else 1,
                                  space="PSUM") as opsum, \
                     tc.tile_pool(name="tpsum", bufs=2 if masked else 3,
                                  space="PSUM") as tpsum:

                    for qc in range(ch_n):  # q chunk of `ch` columns
                        finals = []
                        for qb in range(qb_n):
                            fin = finalp.tile([P, d], F32, tag="final", name=f"fin_{qc}_{qb}")
                            finals.append(fin)
                        for h2 in range(h // hp_n):  # head pairs
                            # scores + exp: full-K128 matmuls against the
                            # zero-padded per-head kp tiles (no tile_position)
                            exp_tiles = {}
                            for hp in range(hp_n):
                                hh = h2 * hp_n + hp
                                for kt2 in range(st_n // kt_pair):
                                    sc_ps = spsum.tile([P, kt_pair * ch], F32,
                                                       tag="spsum")
                                    for j in range(kt_pair):
                                        kt = kt2 * kt_pair + j
                                        nc.tensor.matmul(
                                            sc_ps[:, j * ch:(j + 1) * ch],
                                            kp_tiles[hh][:, kt * P:(kt + 1) * P],
                                            qp_tiles[h2][:, qc * ch:(qc + 1) * ch],
                                            start=True,
                                            stop=True,
                                        )
                                    et = expp.tile([P, kt_pair * ch], F32R,
                                                   tag="exp")
                                    if masked:
                                        nc.scalar.activation(
                                            et,
                                            sc_ps,
                                            mybir.ActivationFunctionType.Exp,
                                            bias=mb_t[:, kt2:kt2 + 1],
                                            scale=1.0 / float(np.sqrt(dh)),
                                        )
                                    else:
                                        nc.scalar.activation(
                                            et,
                                            sc_ps,
                                            mybir.ActivationFunctionType.Exp,
                                            scale=1.0 / float(np.sqrt(dh)),
                                        )
                                    for j in range(kt_pair):
                                        exp_tiles[(hh, kt2 * kt_pair + j)] = \
                                            et[:, j * ch:(j + 1) * ch]
                            # AV + normalize per head of the pair
                            for hp in range(hp_n):
                                hh = h2 * hp_n + hp
                                ot_ps = opsum.tile([dh + 1, ch], F32, tag="opsum")
                                for kt in range(st_n):
                                    nc.tensor.matmul(
                                        ot_ps,
                                        vaug_tiles[kt][
                                            :, hh * (dh + 1):(hh + 1) * (dh + 1)
                                        ],
                                        exp_tiles[(hh, kt)],
                                        start=(kt == 0),
                                        stop=(kt == st_n - 1),
                                    )
                                ots = otsp.tile([dh + 1, ch], F32R, tag="ots")
                                hc = ch // 2
                                nc.vector.tensor_copy(ots[:, 0:hc],
                                                      ot_ps[:, 0:hc])
                                nc.vector.tensor_copy(ots[:, hc:ch],
                                                      ot_ps[:, hc:ch])
                                for qb in range(qb_n):
                                    # transpose via REGULAR fp32r matmul with
                                    # identity; fp32r needs even N -> dh+2
                                    tr = tpsum.tile([P, dh + 2], F32, tag="tpsum")
                                    nc.tensor.matmul(
                                        tr,
                                        ots[:, qb * P:(qb + 1) * P],
                                        idr[0:dh + 1, 0:dh + 2],
                                        start=True,
                                        stop=True,
                                    )
                                    rcp = rcpp.tile([P, 1], F32, tag="rcp")
                                    nc.vector.reciprocal(rcp, tr[:, dh:dh + 1])
                                    nc.vector.tensor_scalar_mul(
                                        finals[qb][:, hh * dh:(hh + 1) * dh],
                                        tr[:, 0:dh],
                                        rcp,
                                    )
                            # stream this head-pair's columns out now so the
                            # final store isn't serialized at the kernel tail
                            c0, c1 = h2 * hp_n * dh, (h2 + 1) * hp_n * dh
                            for qb in range(qb_n):
                                row0 = qc * ch + qb * P
                                nc.sync.dma_start(
                                    out=outd[row0:row0 + P, c0:c1],
                                    in_=finals[qb][:, c0:c1],
                                )

    return nc


_CACHE = {}


def _get_compiled(masked=False):
    key = ("nc", masked)
    if key not in _CACHE:
        nc = build_bass(masked=masked)
        nc.compile()
        _CACHE[key] = nc
    return _CACHE[key]


def kernel(q, k, v, mask, Wq, bq, Wk, bk, Wv, bv):
    q = np.asarray(q, dtype=np.float32)
    k = np.asarray(k, dtype=np.float32)
    v = np.asarray(v, dtype=np.float32)
    mask = np.asarray(mask, dtype=np.float32)
    Wq = np.asarray(Wq, dtype=np.float32)
    Wk = np.asarray(Wk, dtype=np.float32)
    Wv = np.asarray(Wv, dtype=np.float32)
    bq = np.asarray(bq, dtype=np.float32)
    bk = np.asarray(bk, dtype=np.float32)
    bv = np.asarray(bv, dtype=np.float32)

    masked = not bool(np.all(mask == 1.0))
    nc = _get_compiled(masked=masked)

    ot_n = D // P
    st_n = S // P
    # shared (per-core identical) host-side layout prep
    wqT = np.ascontiguousarray(Wq.T)
    wkT = np.ascontiguousarray(Wk.T)
    wvT = np.ascontiguousarray(Wv.T)
    bqT = np.ascontiguousarray(bq.reshape(ot_n, P).T)
    bkT = np.ascontiguousarray(bk.reshape(ot_n, P).T)
    # [bv head-slice | 1.0] per head, broadcast across partitions
    bv_aug = np.concatenate(
        [np.concatenate([bv.reshape(H, DH), np.ones((H, 1), np.float32)], axis=1).reshape(-1)]
    ).astype(np.float32)
    bvB = np.ascontiguousarray(np.broadcast_to(bv_aug, (P, H * (DH + 1))))

    in_maps = []
    for b in range(B):
        mbias = (-10000.0 * (1.0 - mask[b])).astype(np.float32)
        in_maps.append({
            "qT": np.ascontiguousarray(q[b].T),
            "kT": np.ascontiguousarray(k[b].T),
            "vT": np.ascontiguousarray(v[b].T),
            "wqT": wqT,
            "wkT": wkT,
            "wvT": wvT,
            "bqT": bqT,
            "bkT": bkT,
            "bvB": bvB,
            "mb": np.ascontiguousarray(mbias.reshape(st_n, P).T),
        })

    _CACHE["in_maps"] = in_maps
    res = run_bass_kernel_spmd(nc, in_maps, core_ids=list(range(N_CORES)))
    out = np.stack([res.results[b]["out"] for b in range(B)], axis=0)
    return out.astype(np.float32)
